# revision 9
# baseline (speedup 1.0000x reference)
"""DG-block (dual graph-conv) Trainium2 kernel — nn_DG_Block, v2.

Reference per batch item b (B=8, C=128, N=2000, K=9):
  idx1 = top9(knn keys on features_b); idx2 = top9(... motion_b)
  gf_i = graph_feature(features_b, idx_i) -> [2C, N, 9]
  f_i  = conv_bn_relu(1x3 stride 3) -> conv_bn_relu(1x3) on gf_i
  out_b = f1 + delta * f2        [C, N, 1]
BatchNorm pools over the WHOLE batch -> stats all-reduced across cores.
Sharding: one batch item per NeuronCore (8 cores); params replicated.

Algebra (per branch; w1 [C,2C,1,3] split A_d/B_d; conv biases cancel in BN):
  conv1[o,n,t] = (P x_n)[o] + sum_d y_{d}[idx[n,3t+d]][o],  y_d = -B_d x,
  P = sum_d A_d+B_d; window 0's self tap folds into u = (P-B_0) x, so
  windows need (u|v|v) + gathered taps j=1..8 (v = P x).
  knn rank key: <x_i,x_j> - |x_j|^2/2; rank-1 is the point itself.

v2 design (vs 501us baseline):
  * pd matmul stays EXACT fp32 (measured: any input rounding -> 4.5% err).
    The -|x_j|^2/2 row is accumulated INTO the pd PSUM via a K=2 fp16
    (hi/lo split, residual ~1.5e-5) matmul -> kills the [C,N] negsq
    materialization and 140us of DVE/GpSimd adds.
  * top-8 scan (DVE max8+find_index8) reads pd straight from PSUM
    ([C,2048] tiles, ping-pong over all 8 banks) -> kills 68us of ACT
    PSUM->SBUF copies.
  * value path (tables/conv2/activations) runs fp32r / fp16 (host-sim:
    2.6e-3 rel err worst case at bf16 grade, gate is 2e-2).
  * gathers: fp16 ytab rows (256B), point-major (transpose-mode gather
    crashes this ucode build); half the gather bytes of the baseline.
    Window sums run on DVE/GpSimd during kNN-m; the point->channel PE
    transposes (fp16, 1cyc/row) run post-kNN when PSUM banks free up.
  * index wrap built per 8-chunk group so gathers start mid-kNN.
  * collectives: dummy warm-up AllReduce at t~0 (first CC op pays ~25us
    cold start), 4 small ARs with only the last one exposed in the tail.
"""

import numpy as np

import concourse.bacc as bacc
import concourse.bass as bass
import concourse.mybir as mybir
import concourse.tile as tile
import concourse.bass_utils as bass_utils
from concourse.masks import make_identity

F32 = mybir.dt.float32
F32R = mybir.dt.float32r
F16 = mybir.dt.float16
U32 = mybir.dt.uint32
I16 = mybir.dt.int16
AF = mybir.ActivationFunctionType
ALU = mybir.AluOpType

B = 8
C = 128
N = 2000
EPS = 1e-5
NEG_BIG = -1.0e30

CHUNKS = [(i * 128, min(128, N - i * 128)) for i in range((N + 127) // 128)]
NCH = len(CHUNKS)  # 16
JT = [(j * 512, min(512, N - j * 512)) for j in range(4)]
NGRP = 2  # idx-wrap build granularity: 8 chunks per group

# single 2000-col matmuls are invalid ISA (s3d3_mm_num_elements <= 512)
WIDE_MM = False


def build_kernel(delta_nonneg: bool):
    nc = bacc.Bacc(
        "TRN2",
        target_bir_lowering=False,
        debug=False,
        enable_asserts=False,
        num_devices=B,
        num_swdge_queues=4,
    )

    feat_in = nc.dram_tensor("feat", [C, N], F32, kind="ExternalInput").ap()
    mot_in = nc.dram_tensor("mot", [C, N], F32, kind="ExternalInput").ap()
    wb = {}
    for br in (1, 2):
        wb[br] = {
            "nut": nc.dram_tensor(f"nut{br}", [C, 4 * C], F16, kind="ExternalInput").ap(),
            "vt": nc.dram_tensor(f"vt{br}", [C, C], F16, kind="ExternalInput").ap(),
            "w2t": nc.dram_tensor(f"w2t{br}", [C, 3 * C], F16, kind="ExternalInput").ap(),
            "bn": nc.dram_tensor(f"bn{br}", [C, 4], F32, kind="ExternalInput").ap(),
        }
    delta_in = nc.dram_tensor("delta", [1, 1], F32, kind="ExternalInput").ap()
    out_t = nc.dram_tensor("out", [C, N], F32, kind="ExternalOutput").ap()

    with tile.TileContext(nc) as tc:
        _emit(nc, tc, feat_in, mot_in, wb, delta_in, out_t, delta_nonneg)
    nc.compile()
    return nc


def _emit(nc, tc, feat_in, mot_in, wb, delta_in, out_t, delta_nonneg):
    import contextlib

    ctx = contextlib.ExitStack()
    with ctx:
        sb = ctx.enter_context(tc.tile_pool(name="sb", bufs=1))
        dr = ctx.enter_context(tc.tile_pool(name="dr", bufs=1, space="DRAM"))

        # ---------------- persistent on-chip data ----------------
        x = sb.tile([C, N], F32, name="x")
        nc.sync.dma_start(out=x[:], in_=feat_in)
        m = sb.tile([C, N], F32, name="m")
        nc.sync.dma_start(out=m[:], in_=mot_in)

        xh = sb.tile([C, N], F16, name="xh")
        nc.scalar.activation(out=xh[:], in_=x[:], func=AF.Copy)

        ident = sb.tile([C, C], F32, name="ident")
        make_identity(nc, ident[:])
        ineg = sb.tile([C, C], F32, name="ineg")
        nc.scalar.activation(out=ineg[:], in_=ident[:], func=AF.Copy, scale=NEG_BIG)
        identh = sb.tile([C, C], F16, name="identh")
        nc.scalar.activation(out=identh[:], in_=ident[:], func=AF.Copy)
        ones1 = sb.tile([1, C], F32, name="ones1")
        nc.vector.memset(ones1[:], 1.0)
        ones2 = sb.tile([2, C], F16, name="ones2")
        nc.vector.memset(ones2[:], 1.0)
        neghalfc = sb.tile([C, 1], F32, name="neghalfc")
        nc.vector.memset(neghalfc[:], -0.5)

        w = {}
        for br in (1, 2):
            nut = sb.tile([C, 4 * C], F16, name=f"nut{br}")
            nc.sync.dma_start(out=nut[:], in_=wb[br]["nut"])
            vt = sb.tile([C, C], F16, name=f"vt{br}")
            nc.sync.dma_start(out=vt[:], in_=wb[br]["vt"])
            w2t = sb.tile([C, 3 * C], F16, name=f"w2t{br}")
            nc.sync.dma_start(out=w2t[:], in_=wb[br]["w2t"])
            bn = sb.tile([C, 4], F32, name=f"bn{br}")
            nc.sync.dma_start(out=bn[:], in_=wb[br]["bn"])
            w[br] = dict(nut=nut, vt=vt, w2t=w2t, bn=bn)

        delta_sb = sb.tile([1, 1], F32, name="delta_sb")
        nc.sync.dma_start(out=delta_sb[:], in_=delta_in)

        ytab = {br: dr.tile([N, 3 * C], F16, name=f"ytab{br}") for br in (1, 2)}
        yuv = {
            br: [
                sb.tile([C, 640], F16, name=f"yuv_{br}_{ci}", tag=f"yuv{br}", bufs=NCH)
                for ci in range(NCH)
            ]
            for br in (1, 2)
        }
        idx8 = {s: sb.tile([C, NCH * 8], U32, name=f"idx8_{s}") for s in (1, 2)}
        idx9 = {s: sb.tile([C, NCH * 8], U32, name=f"idx9_{s}") for s in (1, 2)}
        idxw = {s: sb.tile([C, NCH * 64], I16, name=f"idxw_{s}") for s in (1, 2)}
        negsq2 = {s: sb.tile([2, N], F16, name=f"negsq2_{s}") for s in (1, 2)}
        o1 = {br: sb.tile([C, 3 * N], F16, name=f"o1_{br}") for br in (1, 2)}
        o2 = {br: sb.tile([C, N], F16, name=f"o2_{br}") for br in (1, 2)}
        f1t = sb.tile([C, N], F16, name="f1t")

        pat8 = sb.tile([C, 8], U32, name="pat8")
        for j in range(1, 9):
            nc.vector.memset(pat8[:, j - 1 : j], j % 3)

        # conv1 stats: 1 col/chunk (accumulated at the o1 PSUM copy)
        s1c = {br: sb.tile([C, NCH], F32, name=f"s1c{br}") for br in (1, 2)}
        s2c = {br: sb.tile([C, NCH], F32, name=f"s2c{br}") for br in (1, 2)}
        s1d = {br: sb.tile([C, 4], F32, name=f"s1d{br}") for br in (1, 2)}
        s2d = {br: sb.tile([C, 4], F32, name=f"s2d{br}") for br in (1, 2)}

        # ---------------- collective helpers ----------------
        def allreduce(arq, k, name):
            ar_in = dr.tile([C, k], F32, name=f"arin{name}")
            ar_out = dr.tile([C, k], F32, name=f"arout{name}", addr_space="Shared")
            nc.sync.dma_start(out=ar_in[:], in_=arq[:])
            nc.gpsimd.collective_compute(
                "AllReduce",
                ALU.add,
                replica_groups=[list(range(B))],
                ins=[ar_in[:].opt()],
                outs=[ar_out[:].opt()],
            )
            art = sb.tile([C, k], F32, name=f"art{name}")
            nc.sync.dma_start(out=art[:], in_=ar_out[:])
            return art

        # warm-up: first CC op pays ~25us stream cold-start + barrier; burn it
        # at t~0 on a dummy payload so the real ARs run warm.
        warm = sb.tile([C, 2], F32, name="warm")
        nc.vector.memset(warm[:], 0.0)
        allreduce(warm, 2, "wu")

        def affine_from(art, col, m_count, br, bn_cols, name):
            inv_m = 1.0 / float(m_count)
            gcol = w[br]["bn"][:, bn_cols[0] : bn_cols[0] + 1]
            bcol = w[br]["bn"][:, bn_cols[1] : bn_cols[1] + 1]
            mean = sb.tile([C, 1], F32, name=f"mean{name}")
            nc.vector.tensor_scalar_mul(mean[:], art[:, col : col + 1], inv_m)
            ey2 = sb.tile([C, 1], F32, name=f"ey2{name}")
            nc.vector.tensor_scalar_mul(ey2[:], art[:, col + 1 : col + 2], inv_m)
            var = sb.tile([C, 1], F32, name=f"var{name}")
            nc.vector.tensor_tensor(out=var[:], in0=mean[:], in1=mean[:], op=ALU.mult)
            nc.vector.tensor_tensor(out=var[:], in0=ey2[:], in1=var[:], op=ALU.subtract)
            nc.vector.tensor_scalar_add(var[:], var[:], EPS)
            rv = sb.tile([C, 1], F32, name=f"rv{name}")
            nc.vector.reciprocal(rv[:], var[:])
            rstd = sb.tile([C, 1], F32, name=f"rstd{name}")
            nc.scalar.activation(out=rstd[:], in_=rv[:], func=AF.Sqrt)
            a_col = sb.tile([C, 1], F32, name=f"acol{name}")
            nc.vector.tensor_tensor(out=a_col[:], in0=gcol, in1=rstd[:], op=ALU.mult)
            c_col = sb.tile([C, 1], F32, name=f"ccol{name}")
            nc.vector.tensor_tensor(out=c_col[:], in0=mean[:], in1=a_col[:], op=ALU.mult)
            nc.vector.tensor_tensor(out=c_col[:], in0=bcol, in1=c_col[:], op=ALU.subtract)
            return (a_col, c_col)

        # ================ phase 0: tables + negsq prep (st psum) ==========
        dcol = sb.tile([C, 1], F32, name="dcol")
        with tc.tile_pool(name="st0", bufs=2, space="PSUM") as st:
            dps = st.tile([C, 8], F32, name="dps", tag="st")
            nc.tensor.matmul(
                out=dps[:, 0:1], lhsT=ones1[:], rhs=delta_sb[0:1, 0:1],
                start=True, stop=True,
            )
            nc.scalar.activation(out=dcol[:], in_=dps[:, 0:1], func=AF.Copy)

            # negsq rows (exact fp32 -> fp16 hi/lo split) per source
            for s, src in ((1, x), (2, m)):
                xsq = sb.tile([C, N], F32, name=f"xsq_{s}", tag="xsq", bufs=1)
                nc.scalar.activation(out=xsq[:], in_=src[:], func=AF.Square)
                sqrow = sb.tile([1, N], F32, name=f"sqrow_{s}", tag="sqrow", bufs=1)
                for j0, jn in JT:
                    sqps = st.tile([1, 512], F32, name=f"sqps_{s}_{j0}", tag="st")
                    nc.tensor.matmul(
                        out=sqps[0:1, :jn], lhsT=neghalfc[:], rhs=xsq[:, j0 : j0 + jn],
                        start=True, stop=True,
                    )
                    nc.scalar.activation(
                        out=sqrow[0:1, j0 : j0 + jn], in_=sqps[0:1, :jn], func=AF.Copy
                    )
                nc.scalar.activation(
                    out=negsq2[s][0:1, :], in_=sqrow[0:1, :], func=AF.Copy
                )
                hi32 = sb.tile([1, N], F32, name=f"hi32_{s}", tag="hi32", bufs=1)
                nc.scalar.activation(
                    out=hi32[0:1, :], in_=negsq2[s][0:1, :], func=AF.Copy
                )
                lo32 = sb.tile([1, N], F32, name=f"lo32_{s}", tag="lo32", bufs=1)
                nc.vector.tensor_tensor(
                    out=lo32[0:1, :], in0=sqrow[0:1, :], in1=hi32[0:1, :],
                    op=ALU.subtract,
                )
                # engines can't address base partition 1; bounce via DMA
                lo16row = sb.tile([1, N], F16, name=f"lo16_{s}", tag="lo16", bufs=1)
                nc.scalar.activation(
                    out=lo16row[0:1, :], in_=lo32[0:1, :], func=AF.Copy
                )
                nc.sync.dma_start(out=negsq2[s][1:2, :], in_=lo16row[0:1, :])

            # per-chunk point-major tables: [y0|y1|y2|u|v] in one go.
            # cols 0:384 -> ytab DRAM rows; 384:512 = u; 512:640 = v.
            for br in (1, 2):
                for ci, (c0, cn) in enumerate(CHUNKS):
                    yp1 = st.tile([C, 512], F32, name=f"yp1_{br}_{ci}", tag="st")
                    nc.tensor.matmul(
                        out=yp1[:cn, :],
                        lhsT=xh[:, c0 : c0 + cn],
                        rhs=w[br]["nut"][:],
                        start=True, stop=True,
                    )
                    yp2 = st.tile([C, 128], F32, name=f"yp2_{br}_{ci}", tag="st")
                    nc.tensor.matmul(
                        out=yp2[:cn, :],
                        lhsT=xh[:, c0 : c0 + cn],
                        rhs=w[br]["vt"][:],
                        start=True, stop=True,
                    )
                    yuvt = yuv[br][ci]
                    nc.scalar.activation(
                        out=yuvt[:cn, 0:512], in_=yp1[:cn, :], func=AF.Copy
                    )
                    nc.scalar.activation(
                        out=yuvt[:cn, 512:640], in_=yp2[:cn, :], func=AF.Copy
                    )
                    nc.sync.dma_start(
                        out=ytab[br][c0 : c0 + cn, :], in_=yuvt[:cn, 0:384]
                    )

        # ---------------- kNN chunk emitters (pd psum) ----------------
        d2 = {s: dr.tile([16, NCH * 64], I16, name=f"ibounce_{s}") for s in (1, 2)}

        def knn_chunk(pd_pool, src, which, ci):
            c0, cn = CHUNKS[ci]
            pps = pd_pool.tile([C, 2048], F32, name=f"pps_{which}_{ci}", tag="pd")
            if WIDE_MM:
                nc.tensor.matmul(
                    out=pps[:cn, 0:N],
                    lhsT=ones2[0:2, 0:cn],
                    rhs=negsq2[which][0:2, 0:N],
                    start=True, stop=False, skip_group_check=True,
                )
                nc.tensor.matmul(
                    out=pps[:cn, 0:N],
                    lhsT=src[:, c0 : c0 + cn],
                    rhs=src[:, 0:N],
                    start=False, stop=True, skip_group_check=True,
                )
            else:
                for j0, jn in JT:
                    nc.tensor.matmul(
                        out=pps[:cn, j0 : j0 + jn],
                        lhsT=ones2[0:2, 0:cn],
                        rhs=negsq2[which][0:2, j0 : j0 + jn],
                        start=True, stop=False, skip_group_check=True,
                    )
                    nc.tensor.matmul(
                        out=pps[:cn, j0 : j0 + jn],
                        lhsT=src[:, c0 : c0 + cn],
                        rhs=src[:, j0 : j0 + jn],
                        start=False, stop=True, skip_group_check=True,
                    )
            # self-exclusion: push the diagonal out of the top-8
            nc.vector.tensor_tensor(
                out=pps[:cn, c0 : c0 + cn],
                in0=pps[:cn, c0 : c0 + cn],
                in1=ineg[:cn, :cn],
                op=ALU.add,
            )
            v8 = sb.tile([C, 8], F32, name=f"v8_{which}_{ci}", tag="v8", bufs=2)
            nc.vector.max(out=v8[:cn], in_=pps[:cn, 0:N])
            nc.vector.max_index(
                out=idx8[which][:cn, ci * 8 : ci * 8 + 8],
                in_max=v8[:cn],
                in_values=pps[:cn, 0:N],
            )

        def build_idx9_group(which, g):
            # cols of idx8/idx9 for chunks [g*8, (g+1)*8): 64 (ci,j) columns
            lo, hi = g * 64, (g + 1) * 64
            v = idx9[which][:, lo:hi].rearrange("p (ci j) -> p ci j", j=8)
            i8 = idx8[which][:, lo:hi].rearrange("p (ci j) -> p ci j", j=8)
            nc.vector.tensor_scalar_mul(v, i8, 3)
            p8 = pat8[:, 0:8].rearrange("p (x j) -> p x j", x=1)
            p8b, _ = bass.broadcast_tensor_aps(p8, v)
            nc.vector.tensor_tensor(out=v, in0=v, in1=p8b, op=ALU.add)
            nc.vector.tensor_scalar_min(
                idx9[which][:, lo:hi], idx9[which][:, lo:hi], 3 * N - 1
            )
            # u16 low-half extract, X-bar transpose (padded to 128 cols: the
            # X-bar tile needs 128-divisible dims), wrap shuffle, DRAM bounce
            loc = sb.tile([C, C], I16, name=f"loc_{which}_{g}", tag="loc", bufs=2)
            nc.vector.memset(loc[:, 64:128], 0)
            lo16 = idx9[which][:, lo:hi].bitcast(I16).rearrange(
                "p (c two) -> p c two", two=2
            )[:, :, 0]
            nc.vector.tensor_tensor(out=loc[:, 0:64], in0=lo16, in1=lo16, op=ALU.bypass)
            tt = sb.tile([C, C], I16, name=f"tt_{which}_{g}", tag="tt", bufs=2)
            nc.sync.dma_start_transpose(out=tt[:], in_=loc[:])
            tt2 = sb.tile([64, C], I16, name=f"tt2_{which}_{g}", tag="tt2", bufs=2)
            dstv = tt2[:, 0:C].rearrange("q (rr ph) -> q rr ph", rr=16)
            srcv = tt[0:64, 0:C].rearrange("q (ph rr) -> q ph rr", ph=8).rearrange(
                "q ph rr -> q rr ph"
            )
            nc.vector.tensor_tensor(out=dstv, in0=srcv, in1=srcv, op=ALU.bypass)
            d2s = d2[which][:, g * 512 : (g + 1) * 512]
            d2v = d2s.rearrange("rr (cj ph) -> cj rr ph", cj=64, ph=8)
            nc.sync.dma_start(
                out=d2v, in_=tt2[:, 0:C].rearrange("q (rr ph) -> q rr ph", rr=16)
            )
            for k in range(8):
                nc.sync.dma_start(
                    out=idxw[which][16 * k : 16 * k + 16, g * 512 : (g + 1) * 512],
                    in_=d2s,
                )

        # ---------------- gather + conv1 (no psum) ----------------
        g9tiles = {}

        def gather_chunk(br, which, ci):
            g9t = sb.tile([C, 1024], F16, name=f"g9_{br}_{ci}", tag="g9", bufs=6)
            ytab3 = ytab[br][:, :].rearrange("n (d c) -> (n d) c", d=3)
            nc.gpsimd.dma_gather(
                out_ap=g9t[:, 0:1024].rearrange("p (q e) -> p q e", q=8),
                in_ap=ytab3,
                idxs_ap=idxw[which][:, ci * 64 : ci * 64 + 64],
                num_idxs=1024,
                num_idxs_reg=1024,
                elem_size=C,
                queue_num=ci % 4,
            )
            g9tiles[(br, ci)] = g9t

        g3tiles = {}

        def conv1_post(br, ci):
            # point-major window sums: g3[n, t*C+c] = base + 2-3 gathered taps
            c0, cn = CHUNKS[ci]
            g9t = g9tiles.pop((br, ci))
            g3 = sb.tile([C, 384], F16, name=f"g3_{br}_{ci}", tag=f"g3{br}", bufs=NCH)
            yuvt = yuv[br][ci]
            # window 0: u + g(j=1) + g(j=2)
            nc.vector.tensor_tensor(
                out=g3[:cn, 0:C], in0=g9t[:cn, 0:C], in1=g9t[:cn, C : 2 * C],
                op=ALU.add,
            )
            nc.vector.tensor_tensor(
                out=g3[:cn, 0:C], in0=g3[:cn, 0:C], in1=yuvt[:cn, 384:512],
                op=ALU.add,
            )
            # windows 1,2: v + sum_d g(j=3t+d)
            g12 = g9t[:, 256:1024].rearrange("p (t d c) -> p t d c", t=2, d=3)
            w12 = g3[:, C : 3 * C].rearrange("p (t c) -> p t c", t=2)
            nc.gpsimd.tensor_tensor(
                out=w12[:cn], in0=g12[:cn, :, 0, :], in1=g12[:cn, :, 1, :], op=ALU.add
            )
            nc.gpsimd.tensor_tensor(
                out=w12[:cn], in0=w12[:cn], in1=g12[:cn, :, 2, :], op=ALU.add
            )
            vsl = yuvt[:, 512:640].rearrange("p (t c) -> p t c", t=1)[:cn]
            vb, _ = bass.broadcast_tensor_aps(vsl, w12[:cn])
            nc.vector.tensor_tensor(out=w12[:cn], in0=w12[:cn], in1=vb, op=ALU.add)
            g3tiles[(br, ci)] = g3

        def transpose_chunk(tp_pool, br, ci):
            # point->channel fp16 PE transposes + o1 store + BN stats
            c0, cn = CHUNKS[ci]
            g3 = g3tiles.pop((br, ci))
            tps = tp_pool.tile([C, 384], F16, name=f"tps_{br}_{ci}", tag="tp")
            for t in range(3):
                nc.tensor.matmul(
                    out=tps[:, t * C : t * C + cn],
                    lhsT=g3[:cn, t * C : t * C + C],
                    rhs=identh[:cn, :cn],
                    is_transpose=True,
                    start=True, stop=True,
                    skip_group_check=True,
                )
            src_ap = tps[:, 0:384].rearrange("p (t n) -> p t n", t=3)[:, :, :cn]
            o1v = o1[br][:, 0 : 3 * N].rearrange("p (t n) -> p t n", t=3)[
                :, :, c0 : c0 + cn
            ]
            nc.scalar.activation(
                out=o1v, in_=src_ap, func=AF.Copy,
                accum_out=s1c[br][:, ci : ci + 1],
            )
            osq = sb.tile([C, 3 * 128], F16, name=f"osq_{br}_{ci}", tag="osq", bufs=2)
            nc.scalar.activation(
                out=osq[:, 0 : 3 * 128].rearrange("p (t n) -> p t n", t=3)[:, :, :cn],
                in_=src_ap,
                func=AF.Square,
                accum_out=s2c[br][:, ci : ci + 1],
            )

        # ================ phase 1: kNN + gathers + conv1 sums =============
        with tc.tile_pool(name="pd", bufs=2, space="PSUM") as pd_pool:
            for ci in range(NCH):
                knn_chunk(pd_pool, x, 1, ci)
                if ci == 7:
                    build_idx9_group(1, 0)
            build_idx9_group(1, 1)
            for ci in range(NCH):
                knn_chunk(pd_pool, m, 2, ci)
                gather_chunk(1, 1, ci)
                if ci >= 2:
                    conv1_post(1, ci - 2)
                if ci == 7:
                    build_idx9_group(2, 0)
            build_idx9_group(2, 1)
            conv1_post(1, 14)
            conv1_post(1, 15)
            for ci in range(NCH):
                gather_chunk(2, 2, ci)
                if ci >= 2:
                    conv1_post(2, ci - 2)
            conv1_post(2, 14)
            conv1_post(2, 15)

        # ========= phase 1b: transposes + conv1 stats + AR1/AR2 ===========
        with tc.tile_pool(name="tp", bufs=4, space="PSUM") as tp_pool:
            for ci in range(NCH):
                transpose_chunk(tp_pool, 1, ci)
            arq1 = sb.tile([C, 2], F32, name="arq1")
            nc.vector.reduce_sum(
                out=arq1[:, 0:1], in_=s1c[1][:], axis=mybir.AxisListType.X
            )
            nc.vector.reduce_sum(
                out=arq1[:, 1:2], in_=s2c[1][:], axis=mybir.AxisListType.X
            )
            art1 = allreduce(arq1, 2, "1")
            for ci in range(NCH):
                transpose_chunk(tp_pool, 2, ci)
            arq2 = sb.tile([C, 2], F32, name="arq2")
            nc.vector.reduce_sum(
                out=arq2[:, 0:1], in_=s1c[2][:], axis=mybir.AxisListType.X
            )
            nc.vector.reduce_sum(
                out=arq2[:, 1:2], in_=s2c[2][:], axis=mybir.AxisListType.X
            )
            art2 = allreduce(arq2, 2, "2")

        # ================ phase 2: conv2 + final ARs + merge ==============
        with tc.tile_pool(name="st1", bufs=4, space="PSUM") as st:

            def conv2_branch(br, aff):
                a_col, c_col = aff
                o1t = o1[br]
                for jt, (j0, jn) in enumerate(JT):
                    o1v = o1t[:, 0 : 3 * N].rearrange("p (t n) -> p t n", t=3)[
                        :, :, j0 : j0 + jn
                    ]
                    nc.scalar.activation(
                        out=o1v, in_=o1v, func=AF.Relu, scale=a_col[:], bias=c_col[:]
                    )
                    ps = st.tile([C, 512], F32, name=f"o2ps_{br}_{jt}", tag="c2")
                    for d in range(3):
                        nc.tensor.matmul(
                            out=ps[:, :jn],
                            lhsT=w[br]["w2t"][:, d * C : (d + 1) * C],
                            rhs=o1t[:, d * N + j0 : d * N + j0 + jn],
                            start=(d == 0), stop=(d == 2),
                        )
                    nc.scalar.activation(
                        out=o2[br][:, j0 : j0 + jn], in_=ps[:, :jn], func=AF.Copy,
                        accum_out=s1d[br][:, jt : jt + 1],
                    )
                    osq = sb.tile([C, 512], F16, name=f"o2sq_{br}_{jt}", tag="o2sq", bufs=2)
                    nc.scalar.activation(
                        out=osq[:, :jn], in_=ps[:, :jn], func=AF.Square,
                        accum_out=s2d[br][:, jt : jt + 1],
                    )

            aff1_1 = affine_from(art1, 0, B * N * 3, 1, (0, 1), "c1b1")
            conv2_branch(1, aff1_1)
            arq3 = sb.tile([C, 2], F32, name="arq3")
            nc.vector.reduce_sum(
                out=arq3[:, 0:1], in_=s1d[1][:], axis=mybir.AxisListType.X
            )
            nc.vector.reduce_sum(
                out=arq3[:, 1:2], in_=s2d[1][:], axis=mybir.AxisListType.X
            )
            art3 = allreduce(arq3, 2, "3")

            aff1_2 = affine_from(art2, 0, B * N * 3, 2, (0, 1), "c1b2")
            conv2_branch(2, aff1_2)
            arq4 = sb.tile([C, 2], F32, name="arq4")
            nc.vector.reduce_sum(
                out=arq4[:, 0:1], in_=s1d[2][:], axis=mybir.AxisListType.X
            )
            nc.vector.reduce_sum(
                out=arq4[:, 1:2], in_=s2d[2][:], axis=mybir.AxisListType.X
            )
            art4 = allreduce(arq4, 2, "4")

            # f1 while AR4 is in flight
            a1, c1 = affine_from(art3, 0, B * N, 1, (2, 3), "c2b1")
            for j0, jn in JT:
                nc.scalar.activation(
                    out=f1t[:, j0 : j0 + jn], in_=o2[1][:, j0 : j0 + jn],
                    func=AF.Relu, scale=a1[:], bias=c1[:],
                )
            a2, c2 = affine_from(art4, 0, B * N, 2, (2, 3), "c2b2")
            if delta_nonneg:
                a2d = sb.tile([C, 1], F32, name="a2d")
                nc.vector.tensor_tensor(out=a2d[:], in0=a2[:], in1=dcol[:], op=ALU.mult)
                c2d = sb.tile([C, 1], F32, name="c2d")
                nc.vector.tensor_tensor(out=c2d[:], in0=c2[:], in1=dcol[:], op=ALU.mult)
            for j0, jn in JT:
                f2 = sb.tile([C, 512], F16, name=f"f2_{j0}", tag="f2", bufs=2)
                if delta_nonneg:
                    nc.scalar.activation(
                        out=f2[:, :jn], in_=o2[2][:, j0 : j0 + jn],
                        func=AF.Relu, scale=a2d[:], bias=c2d[:],
                    )
                else:
                    nc.scalar.activation(
                        out=f2[:, :jn], in_=o2[2][:, j0 : j0 + jn],
                        func=AF.Relu, scale=a2[:], bias=c2[:],
                    )
                    nc.vector.tensor_scalar_mul(f2[:, :jn], f2[:, :jn], dcol[:])
                of = sb.tile([C, 512], F32, name=f"of_{j0}", tag="of", bufs=2)
                nc.vector.tensor_tensor(
                    out=of[:, :jn], in0=f1t[:, j0 : j0 + jn], in1=f2[:, :jn], op=ALU.add
                )
                nc.sync.dma_start(out=out_t[:, j0 : j0 + jn], in_=of[:, :jn])


# ======================= host side =======================

_CACHE = {}


def _prep_branch(w1, b1, g1, be1, w2, b2, g2, be2):
    w1 = np.asarray(w1, dtype=np.float32)
    w2 = np.asarray(w2, dtype=np.float32)
    A = w1[:, :C, 0, :]  # [o, i, 3]
    Bm = w1[:, C:, 0, :]  # [o, i, 3]
    P = (A + Bm).sum(axis=2)  # [o, i]
    ut = (P - Bm[:, :, 0]).T  # [i, o]: u = (P - B0) x
    vt = np.ascontiguousarray(P.T).astype(np.float16)
    nbt = np.concatenate([(-Bm[:, :, d]).T for d in range(3)], axis=1)  # [i, 3C]
    nut = np.ascontiguousarray(
        np.concatenate([nbt, ut], axis=1)
    ).astype(np.float16)  # [i, 4C] = [y0|y1|y2|u] weights
    w2t = np.ascontiguousarray(
        np.concatenate([w2[:, :, 0, d].T for d in range(3)], axis=1)
    ).astype(np.float16)  # [i, 3C] fp16
    bn = np.ascontiguousarray(
        np.stack(
            [
                np.asarray(g1, np.float32),
                np.asarray(be1, np.float32),
                np.asarray(g2, np.float32),
                np.asarray(be2, np.float32),
            ],
            axis=1,
        )
    )  # [C, 4]
    return nut, vt, w2t, bn


def kernel(**inputs):
    features = np.ascontiguousarray(np.asarray(inputs["features"], np.float32))
    motion = np.ascontiguousarray(np.asarray(inputs["motion"], np.float32))
    delta = np.asarray(inputs["delta"], np.float32).reshape(-1)[0]

    nut1, vt1, w2t1, bn1 = _prep_branch(
        inputs["d1_w1"], inputs["d1_b1"], inputs["d1_g1"], inputs["d1_be1"],
        inputs["d1_w2"], inputs["d1_b2"], inputs["d1_g2"], inputs["d1_be2"],
    )
    nut2, vt2, w2t2, bn2 = _prep_branch(
        inputs["d2_w1"], inputs["d2_b1"], inputs["d2_g1"], inputs["d2_be1"],
        inputs["d2_w2"], inputs["d2_b2"], inputs["d2_g2"], inputs["d2_be2"],
    )

    delta_nonneg = bool(delta >= 0.0)
    key = ("dg2", delta_nonneg)
    if key not in _CACHE:
        _CACHE[key] = build_kernel(delta_nonneg)
    nc = _CACHE[key]

    shared = {
        "nut1": nut1, "vt1": vt1, "w2t1": w2t1, "bn1": bn1,
        "nut2": nut2, "vt2": vt2, "w2t2": w2t2, "bn2": bn2,
        "delta": np.array([[delta]], np.float32),
    }
    in_maps = []
    for c in range(B):
        im = dict(shared)
        im["feat"] = np.ascontiguousarray(features[c, :, :, 0])
        im["mot"] = np.ascontiguousarray(motion[c, :, :, 0])
        in_maps.append(im)

    import os

    trace = bool(int(os.environ.get("DG_KERNEL_TRACE", "0")))
    res = bass_utils.run_bass_kernel_spmd(
        nc, in_maps, core_ids=list(range(B)), trace=trace
    )
    global LAST_RESULTS
    LAST_RESULTS = res
    out = np.stack([res.results[c]["out"] for c in range(B)], axis=0)
    return out.reshape(B, C, N, 1).astype(np.float32)


LAST_RESULTS = None


# revision 13
# speedup vs baseline: 1.5498x; 1.5498x over previous
"""DG-block (dual graph-conv) Trainium2 kernel — nn_DG_Block, v3.

Reference per batch item b (B=8, C=128, N=2000, K=9):
  idx1 = top9(knn keys on features_b); idx2 = top9(... motion_b)
  gf_i = graph_feature(features_b, idx_i) -> [2C, N, 9]
  f_i  = conv_bn_relu(1x3 stride 3) -> conv_bn_relu(1x3) on gf_i
  out_b = f1 + delta * f2        [C, N, 1]
BatchNorm pools over the WHOLE batch -> stats all-reduced across cores.
Sharding: one batch item per NeuronCore (8 cores); params replicated.

Algebra (per branch; w1 [C,2C,1,3] split A_d/B_d; conv biases cancel in BN):
  conv1[o,n,t] = base_t[o,n] + sum_d y_d[idx[n,3t+d]][o],  y_d = -B_d x,
  base_0 = u = (P-B_0) x (self tap folded), base_1 = base_2 = v = P x,
  P = sum_d A_d+B_d.  knn rank key: <x_i,x_j> - |x_j|^2/2.

v3 design (baseline 501us; v2b's 938us taught the hard lessons):
  * pd matmul EXACT fp32 (input rounding -> 4.5% err, gate 2e-2). The
    -|x_j|^2/2 row accumulates INTO pd PSUM via one K=2 fp16 hi/lo
    matmul per 512-tile (residual 1.5e-5) -> kills the [C,N] negsq
    materialization + 140us of DVE/GpSimd adds.
  * PSUM: pd ([C,1024]x4bufs, 4 banks) + st (2) + tp (2) coexist ->
    no pool barriers; tables run INSIDE kNN-f so PE never drops to the
    cold p-state; scans read an SBUF pdt staging copy (ACT does the 2
    copies/chunk, ACT has the headroom).
  * value path fp16 (host-sim 5e-4 rel err; gate 2e-2).
  * gathers: 32x 1024-idx fp16 dma_gather (~4us engine-hold each,
    measured; 2048-idx and transpose-mode crash the ucode). Emitted as
    one dense stream starting mid-kNN-f (branch1 idx group A is ready
    after chunk 7) so the serial GpSimd holds fully overlap kNN.
  * conv1: point-major window sums (DVE+GpSimd) -> 3 fp16 PE
    transposes -> DVE scalar_tensor_tensor fuses the channel-major
    u/v base-add + PSUM->SBUF copy + BN-sum accumulation in 2 ops.
  * collectives: dummy warm-up AllReduce at t~0 (first CC op pays
    ~25us cold-start), 4 small ARs, only the last exposed.
"""

import numpy as np

import concourse.bacc as bacc
import concourse.bass as bass
import concourse.mybir as mybir
import concourse.tile as tile
import concourse.bass_utils as bass_utils
from concourse.masks import make_identity

F32 = mybir.dt.float32
F16 = mybir.dt.float16
U32 = mybir.dt.uint32
I16 = mybir.dt.int16
AF = mybir.ActivationFunctionType
ALU = mybir.AluOpType

B = 8
C = 128
N = 2000
EPS = 1e-5
NEG_BIG = -1.0e30

CHUNKS = [(i * 128, min(128, N - i * 128)) for i in range((N + 127) // 128)]
NCH = len(CHUNKS)  # 16
JT = [(j * 512, min(512, N - j * 512)) for j in range(4)]


def build_kernel(delta_nonneg: bool):
    nc = bacc.Bacc(
        "TRN2",
        target_bir_lowering=False,
        debug=False,
        enable_asserts=False,
        num_devices=B,
        num_swdge_queues=4,
    )

    feat_in = nc.dram_tensor("feat", [C, N], F32, kind="ExternalInput").ap()
    mot_in = nc.dram_tensor("mot", [C, N], F32, kind="ExternalInput").ap()
    wb = {}
    for br in (1, 2):
        wb[br] = {
            "nbt": nc.dram_tensor(f"nbt{br}", [C, 3 * C], F16, kind="ExternalInput").ap(),
            "utc": nc.dram_tensor(f"utc{br}", [C, C], F16, kind="ExternalInput").ap(),
            "vtc": nc.dram_tensor(f"vtc{br}", [C, C], F16, kind="ExternalInput").ap(),
            "w2t": nc.dram_tensor(f"w2t{br}", [C, 3 * C], F16, kind="ExternalInput").ap(),
            "bn": nc.dram_tensor(f"bn{br}", [C, 4], F32, kind="ExternalInput").ap(),
        }
    delta_in = nc.dram_tensor("delta", [1, 1], F32, kind="ExternalInput").ap()
    out_t = nc.dram_tensor("out", [C, N], F32, kind="ExternalOutput").ap()

    with tile.TileContext(nc) as tc:
        _emit(nc, tc, feat_in, mot_in, wb, delta_in, out_t, delta_nonneg)
    nc.compile()
    return nc


def _emit(nc, tc, feat_in, mot_in, wb, delta_in, out_t, delta_nonneg):
    import contextlib

    ctx = contextlib.ExitStack()
    with ctx:
        sb = ctx.enter_context(tc.tile_pool(name="sb", bufs=1))
        dr = ctx.enter_context(tc.tile_pool(name="dr", bufs=1, space="DRAM"))
        # pd: 2 tiles/chunk x 2 chunks in flight = 4 banks
        pd_ps = ctx.enter_context(tc.tile_pool(name="pd", bufs=4, space="PSUM"))
        st_ps = ctx.enter_context(tc.tile_pool(name="st", bufs=2, space="PSUM"))
        tp_ps = ctx.enter_context(tc.tile_pool(name="tp", bufs=2, space="PSUM"))

        # ---------------- persistent on-chip data ----------------
        x = sb.tile([C, N], F32, name="x")
        nc.sync.dma_start(out=x[:], in_=feat_in)
        m = sb.tile([C, N], F32, name="m")
        nc.sync.dma_start(out=m[:], in_=mot_in)

        ident = sb.tile([C, C], F32, name="ident")
        make_identity(nc, ident[:])
        ineg = sb.tile([C, C], F32, name="ineg")
        nc.scalar.activation(out=ineg[:], in_=ident[:], func=AF.Copy, scale=NEG_BIG)
        identh = sb.tile([C, C], F16, name="identh")
        nc.scalar.activation(out=identh[:], in_=ident[:], func=AF.Copy)
        ones1 = sb.tile([1, C], F32, name="ones1")
        nc.vector.memset(ones1[:], 1.0)
        ones2 = sb.tile([2, C], F16, name="ones2")
        nc.vector.memset(ones2[:], 1.0)
        neghalfc = sb.tile([C, 1], F32, name="neghalfc")
        nc.vector.memset(neghalfc[:], -0.5)

        w = {}
        for br in (1, 2):
            nbt = sb.tile([C, 3 * C], F16, name=f"nbt{br}")
            nc.sync.dma_start(out=nbt[:], in_=wb[br]["nbt"])
            utc = sb.tile([C, C], F16, name=f"utc{br}")
            nc.sync.dma_start(out=utc[:], in_=wb[br]["utc"])
            vtc = sb.tile([C, C], F16, name=f"vtc{br}")
            nc.sync.dma_start(out=vtc[:], in_=wb[br]["vtc"])
            w2t = sb.tile([C, 3 * C], F16, name=f"w2t{br}")
            nc.sync.dma_start(out=w2t[:], in_=wb[br]["w2t"])
            bn = sb.tile([C, 4], F32, name=f"bn{br}")
            nc.sync.dma_start(out=bn[:], in_=wb[br]["bn"])
            w[br] = dict(nbt=nbt, utc=utc, vtc=vtc, w2t=w2t, bn=bn)

        delta_sb = sb.tile([1, 1], F32, name="delta_sb")
        nc.sync.dma_start(out=delta_sb[:], in_=delta_in)

        ytab = {br: dr.tile([N, 3 * C], F16, name=f"ytab{br}") for br in (1, 2)}
        idx8 = {s: sb.tile([C, NCH * 8], U32, name=f"idx8_{s}") for s in (1, 2)}
        idx9 = {s: sb.tile([C, NCH * 8], U32, name=f"idx9_{s}") for s in (1, 2)}
        idxw = {s: sb.tile([C, NCH * 64], I16, name=f"idxw_{s}") for s in (1, 2)}
        negsq2 = {s: sb.tile([2, N], F16, name=f"negsq2_{s}") for s in (1, 2)}
        # channel-major conv1 bases: [u plane | v plane]
        uvch = {br: sb.tile([C, 2 * N], F16, name=f"uvch{br}") for br in (1, 2)}
        o1 = {br: sb.tile([C, 3 * N], F16, name=f"o1_{br}") for br in (1, 2)}
        o2 = {br: sb.tile([C, N], F16, name=f"o2_{br}") for br in (1, 2)}
        f1t = sb.tile([C, N], F16, name="f1t")

        pat8 = sb.tile([C, 8], U32, name="pat8")
        for j in range(1, 9):
            nc.vector.memset(pat8[:, j - 1 : j], j % 3)

        # conv1 stats: 2 cols/chunk (w0-op + w12-op accums); conv2: 4 JT cols
        s1c = {br: sb.tile([C, 2 * NCH], F32, name=f"s1c{br}") for br in (1, 2)}
        s2c = {br: sb.tile([C, NCH], F32, name=f"s2c{br}") for br in (1, 2)}
        s1d = {br: sb.tile([C, 4], F32, name=f"s1d{br}") for br in (1, 2)}
        s2d = {br: sb.tile([C, 4], F32, name=f"s2d{br}") for br in (1, 2)}

        # ---------------- collective helpers ----------------
        def allreduce(arq, k, name):
            ar_in = dr.tile([C, k], F32, name=f"arin{name}")
            ar_out = dr.tile([C, k], F32, name=f"arout{name}", addr_space="Shared")
            nc.sync.dma_start(out=ar_in[:], in_=arq[:])
            nc.gpsimd.collective_compute(
                "AllReduce",
                ALU.add,
                replica_groups=[list(range(B))],
                ins=[ar_in[:].opt()],
                outs=[ar_out[:].opt()],
            )
            art = sb.tile([C, k], F32, name=f"art{name}")
            nc.sync.dma_start(out=art[:], in_=ar_out[:])
            return art

        # warm-up: first CC op pays ~25us cold-start; burn it at t~0
        warm = sb.tile([C, 2], F32, name="warm")
        nc.vector.memset(warm[:], 0.0)
        allreduce(warm, 2, "wu")

        def affine_from(art, col, m_count, br, bn_cols, name):
            inv_m = 1.0 / float(m_count)
            gcol = w[br]["bn"][:, bn_cols[0] : bn_cols[0] + 1]
            bcol = w[br]["bn"][:, bn_cols[1] : bn_cols[1] + 1]
            mean = sb.tile([C, 1], F32, name=f"mean{name}")
            nc.vector.tensor_scalar_mul(mean[:], art[:, col : col + 1], inv_m)
            ey2 = sb.tile([C, 1], F32, name=f"ey2{name}")
            nc.vector.tensor_scalar_mul(ey2[:], art[:, col + 1 : col + 2], inv_m)
            var = sb.tile([C, 1], F32, name=f"var{name}")
            nc.vector.tensor_tensor(out=var[:], in0=mean[:], in1=mean[:], op=ALU.mult)
            nc.vector.tensor_tensor(out=var[:], in0=ey2[:], in1=var[:], op=ALU.subtract)
            nc.vector.tensor_scalar_add(var[:], var[:], EPS)
            rv = sb.tile([C, 1], F32, name=f"rv{name}")
            nc.vector.reciprocal(rv[:], var[:])
            rstd = sb.tile([C, 1], F32, name=f"rstd{name}")
            nc.scalar.activation(out=rstd[:], in_=rv[:], func=AF.Sqrt)
            a_col = sb.tile([C, 1], F32, name=f"acol{name}")
            nc.vector.tensor_tensor(out=a_col[:], in0=gcol, in1=rstd[:], op=ALU.mult)
            c_col = sb.tile([C, 1], F32, name=f"ccol{name}")
            nc.vector.tensor_tensor(out=c_col[:], in0=mean[:], in1=a_col[:], op=ALU.mult)
            nc.vector.tensor_tensor(out=c_col[:], in0=bcol, in1=c_col[:], op=ALU.subtract)
            return (a_col, c_col)

        # ---------------- startup numerics ----------------
        dcol = sb.tile([C, 1], F32, name="dcol")
        dps = st_ps.tile([C, 8], F32, name="dps", tag="st")
        nc.tensor.matmul(
            out=dps[:, 0:1], lhsT=ones1[:], rhs=delta_sb[0:1, 0:1], start=True, stop=True
        )
        nc.scalar.activation(out=dcol[:], in_=dps[:, 0:1], func=AF.Copy)

        def negsq_prep(s, src):
            # exact fp32 row -|x_j|^2/2 -> fp16 hi/lo pair (residual ~1.5e-5)
            xsq = sb.tile([C, N], F32, name=f"xsq_{s}", tag="xsq", bufs=1)
            nc.scalar.activation(out=xsq[:], in_=src[:], func=AF.Square)
            sqrow = sb.tile([1, N], F32, name=f"sqrow_{s}", tag="sqrow", bufs=1)
            for j0, jn in JT:
                sqps = st_ps.tile([1, 512], F32, name=f"sqps_{s}_{j0}", tag="st")
                nc.tensor.matmul(
                    out=sqps[0:1, :jn], lhsT=neghalfc[:], rhs=xsq[:, j0 : j0 + jn],
                    start=True, stop=True,
                )
                nc.scalar.activation(
                    out=sqrow[0:1, j0 : j0 + jn], in_=sqps[0:1, :jn], func=AF.Copy
                )
            nc.scalar.activation(out=negsq2[s][0:1, :], in_=sqrow[0:1, :], func=AF.Copy)
            hi32 = sb.tile([1, N], F32, name=f"hi32_{s}", tag="hi32", bufs=1)
            nc.scalar.activation(out=hi32[0:1, :], in_=negsq2[s][0:1, :], func=AF.Copy)
            lo32 = sb.tile([1, N], F32, name=f"lo32_{s}", tag="lo32", bufs=1)
            nc.vector.tensor_tensor(
                out=lo32[0:1, :], in0=sqrow[0:1, :], in1=hi32[0:1, :], op=ALU.subtract
            )
            # engines can't address base partition 1; bounce via DMA
            lo16row = sb.tile([1, N], F16, name=f"lo16_{s}", tag="lo16", bufs=1)
            nc.scalar.activation(out=lo16row[0:1, :], in_=lo32[0:1, :], func=AF.Copy)
            nc.sync.dma_start(out=negsq2[s][1:2, :], in_=lo16row[0:1, :])

        negsq_prep(1, x)
        xh = sb.tile([C, N], F16, name="xh")
        nc.scalar.activation(out=xh[:], in_=x[:], func=AF.Copy)
        negsq_prep(2, m)

        # ---------------- per-chunk emitters ----------------
        def tables_chunk(br, ci):
            c0, cn = CHUNKS[ci]
            yps = st_ps.tile([C, 384], F32, name=f"yps_{br}_{ci}", tag="st")
            nc.tensor.matmul(
                out=yps[:cn, :], lhsT=xh[:, c0 : c0 + cn], rhs=w[br]["nbt"][:],
                start=True, stop=True,
            )
            yst = sb.tile([C, 384], F16, name=f"yst_{br}_{ci}", tag="yst", bufs=3)
            nc.scalar.activation(out=yst[:cn, :], in_=yps[:cn, :], func=AF.Copy)
            nc.sync.dma_start(out=ytab[br][c0 : c0 + cn, :], in_=yst[:cn, :])

        def uv_tables(br):
            # channel-major u = (P-B0)x, v = Px (bases added post-transpose)
            for which, lhsw in (("u", w[br]["utc"]), ("v", w[br]["vtc"])):
                off = 0 if which == "u" else N
                for j0, jn in JT:
                    ps = st_ps.tile([C, 512], F32, name=f"uv{br}{which}{j0}", tag="st")
                    nc.tensor.matmul(
                        out=ps[:, :jn], lhsT=lhsw[:], rhs=xh[:, j0 : j0 + jn],
                        start=True, stop=True,
                    )
                    nc.scalar.activation(
                        out=uvch[br][:, off + j0 : off + j0 + jn], in_=ps[:, :jn],
                        func=AF.Copy,
                    )

        def knn_chunk(src, which, ci):
            c0, cn = CHUNKS[ci]
            pdt = sb.tile([C, 2048], F32, name=f"pdt_{which}_{ci}", tag="pdt", bufs=2)
            for sub in range(4):
                j0, jn = JT[sub]
                pps = pd_ps.tile(
                    [C, 512], F32, name=f"pps_{which}_{ci}_{sub}", tag="pd"
                )
                nc.tensor.matmul(
                    out=pps[:cn, 0:jn],
                    lhsT=ones2[0:2, 0:cn],
                    rhs=negsq2[which][0:2, j0 : j0 + jn],
                    start=True, stop=False, skip_group_check=True,
                )
                nc.tensor.matmul(
                    out=pps[:cn, 0:jn],
                    lhsT=src[:, c0 : c0 + cn],
                    rhs=src[:, j0 : j0 + jn],
                    start=False, stop=True, skip_group_check=True,
                )
                nc.scalar.activation(
                    out=pdt[:cn, j0 : j0 + jn], in_=pps[:cn, 0:jn], func=AF.Copy
                )
            # self-exclusion: push the diagonal out of the top-8
            nc.vector.tensor_tensor(
                out=pdt[:cn, c0 : c0 + cn],
                in0=pdt[:cn, c0 : c0 + cn],
                in1=ineg[:cn, :cn],
                op=ALU.add,
            )
            v8 = sb.tile([C, 8], F32, name=f"v8_{which}_{ci}", tag="v8", bufs=2)
            nc.vector.max(out=v8[:cn], in_=pdt[:cn, 0:N])
            nc.vector.max_index(
                out=idx8[which][:cn, ci * 8 : ci * 8 + 8],
                in_max=v8[:cn],
                in_values=pdt[:cn, 0:N],
            )

        d2 = {s: dr.tile([16, NCH * 64], I16, name=f"ibounce_{s}") for s in (1, 2)}

        def build_idx9_group(which, g):
            # chunks [g*8, (g+1)*8) -> wrapped int16 idx table for the ucode
            lo, hi = g * 64, (g + 1) * 64
            v = idx9[which][:, lo:hi].rearrange("p (ci j) -> p ci j", j=8)
            i8 = idx8[which][:, lo:hi].rearrange("p (ci j) -> p ci j", j=8)
            nc.vector.tensor_scalar_mul(v, i8, 3)
            p8 = pat8[:, 0:8].rearrange("p (x j) -> p x j", x=1)
            p8b, _ = bass.broadcast_tensor_aps(p8, v)
            nc.vector.tensor_tensor(out=v, in0=v, in1=p8b, op=ALU.add)
            nc.vector.tensor_scalar_min(
                idx9[which][:, lo:hi], idx9[which][:, lo:hi], 3 * N - 1
            )
            loc = sb.tile([C, C], I16, name=f"loc_{which}_{g}", tag="loc", bufs=2)
            nc.vector.memset(loc[:, 64:128], 0)
            lo16 = idx9[which][:, lo:hi].bitcast(I16).rearrange(
                "p (c two) -> p c two", two=2
            )[:, :, 0]
            nc.vector.tensor_tensor(out=loc[:, 0:64], in0=lo16, in1=lo16, op=ALU.bypass)
            tt = sb.tile([C, C], I16, name=f"tt_{which}_{g}", tag="tt", bufs=2)
            nc.sync.dma_start_transpose(out=tt[:], in_=loc[:])
            tt2 = sb.tile([64, C], I16, name=f"tt2_{which}_{g}", tag="tt2", bufs=2)
            dstv = tt2[:, 0:C].rearrange("q (rr ph) -> q rr ph", rr=16)
            srcv = tt[0:64, 0:C].rearrange("q (ph rr) -> q ph rr", ph=8).rearrange(
                "q ph rr -> q rr ph"
            )
            nc.vector.tensor_tensor(out=dstv, in0=srcv, in1=srcv, op=ALU.bypass)
            d2s = d2[which][:, g * 512 : (g + 1) * 512]
            d2v = d2s.rearrange("rr (cj ph) -> cj rr ph", cj=64, ph=8)
            nc.sync.dma_start(
                out=d2v, in_=tt2[:, 0:C].rearrange("q (rr ph) -> q rr ph", rr=16)
            )
            for k in range(8):
                nc.sync.dma_start(
                    out=idxw[which][16 * k : 16 * k + 16, g * 512 : (g + 1) * 512],
                    in_=d2s,
                )

        g9tiles = {}

        def gather_chunk(br, which, ci):
            g9t = sb.tile([C, 1024], F16, name=f"g9_{br}_{ci}", tag="g9", bufs=10)
            ytab3 = ytab[br][:, :].rearrange("n (d c) -> (n d) c", d=3)
            nc.gpsimd.dma_gather(
                out_ap=g9t[:, 0:1024].rearrange("p (q e) -> p q e", q=8),
                in_ap=ytab3,
                idxs_ap=idxw[which][:, ci * 64 : ci * 64 + 64],
                num_idxs=1024,
                num_idxs_reg=1024,
                elem_size=C,
                queue_num=ci % 4,
            )
            g9tiles[(br, ci)] = g9t

        def conv1_chain(br, ci):
            # point-major window sums -> fp16 transposes -> fused base-add
            c0, cn = CHUNKS[ci]
            g9t = g9tiles.pop((br, ci))
            g3 = sb.tile([C, 384], F16, name=f"g3_{br}_{ci}", tag="g3", bufs=4)
            nc.vector.tensor_tensor(
                out=g3[:cn, 0:C], in0=g9t[:cn, 0:C], in1=g9t[:cn, C : 2 * C],
                op=ALU.add,
            )
            g12 = g9t[:, 256:1024].rearrange("p (t d c) -> p t d c", t=2, d=3)
            w12 = g3[:, C : 3 * C].rearrange("p (t c) -> p t c", t=2)
            nc.gpsimd.tensor_tensor(
                out=w12[:cn], in0=g12[:cn, :, 0, :], in1=g12[:cn, :, 1, :], op=ALU.add
            )
            nc.gpsimd.tensor_tensor(
                out=w12[:cn], in0=w12[:cn], in1=g12[:cn, :, 2, :], op=ALU.add
            )
            tps = tp_ps.tile([C, 384], F16, name=f"tps_{br}_{ci}", tag="tp")
            for t in range(3):
                nc.tensor.matmul(
                    out=tps[:, t * C : t * C + cn],
                    lhsT=g3[:cn, t * C : t * C + C],
                    rhs=identh[:cn, :cn],
                    is_transpose=True,
                    start=True, stop=True,
                    skip_group_check=True,
                )
            # fused: o1 = tps + base, BN-sum accum, PSUM->SBUF, in 2 DVE ops
            o1v = o1[br][:, 0 : 3 * N].rearrange("p (t n) -> p t n", t=3)
            nc.vector.scalar_tensor_tensor(
                out=o1v[:, 0, c0 : c0 + cn],
                in0=tps[:, 0:cn],
                scalar=0.0,
                in1=uvch[br][:, c0 : c0 + cn],
                op0=ALU.add, op1=ALU.add,
                accum_out=s1c[br][:, 2 * ci : 2 * ci + 1],
            )
            vsl = uvch[br][:, N + c0 : N + c0 + cn].rearrange("p (t n) -> p t n", t=1)
            w12t = o1v[:, 1:3, c0 : c0 + cn]
            vb, _ = bass.broadcast_tensor_aps(vsl, w12t)
            nc.vector.scalar_tensor_tensor(
                out=w12t,
                in0=tps[:, 0:384].rearrange("p (t n) -> p t n", t=3)[:, 1:3, :cn],
                scalar=0.0,
                in1=vb,
                op0=ALU.add, op1=ALU.add,
                accum_out=s1c[br][:, 2 * ci + 1 : 2 * ci + 2],
            )
            osq = sb.tile([C, 3 * 128], F16, name=f"osq_{br}_{ci}", tag="osq", bufs=2)
            nc.scalar.activation(
                out=osq[:, 0 : 3 * 128].rearrange("p (t n) -> p t n", t=3)[:, :, :cn],
                in_=o1v[:, :, c0 : c0 + cn],
                func=AF.Square,
                accum_out=s2c[br][:, ci : ci + 1],
            )

        # ================ emission schedule ================
        # kNN-f; tables fill early iterations (PE stays hot); branch-1
        # gathers start as soon as idx group A lands (after chunk 7).
        T1 = {0: [0, 1, 2], 1: [3, 4, 5], 2: [6, 7, 8], 3: [9, 10, 11],
              4: [12, 13], 5: [14, 15]}
        T2 = {6: [0, 1], 7: [2, 3], 8: [4, 5], 9: [6, 7], 10: [8, 9],
              11: [10, 11], 12: [12, 13], 13: [14, 15]}
        for ci in range(NCH):
            knn_chunk(x, 1, ci)
            for t in T1.get(ci, []):
                tables_chunk(1, t)
            if ci == 5:
                uv_tables(1)
            for t in T2.get(ci, []):
                tables_chunk(2, t)
            if ci == 7:
                build_idx9_group(1, 0)
            if ci == 14:
                uv_tables(2)
            if ci >= 8:
                gather_chunk(1, 1, ci - 8)
        build_idx9_group(1, 1)
        for ci in range(NCH):
            knn_chunk(m, 2, ci)
            if ci < 8:
                gather_chunk(1, 1, 8 + ci)
            if ci == 7:
                build_idx9_group(2, 0)
            if ci >= 8:
                gather_chunk(2, 2, ci - 8)
            if ci >= 2:
                conv1_chain(1, ci - 2)
        build_idx9_group(2, 1)
        for k in range(14, NCH):
            conv1_chain(1, k)
        # AR1: branch-1 conv1 stats
        arq1 = sb.tile([C, 2], F32, name="arq1")
        nc.vector.reduce_sum(out=arq1[:, 0:1], in_=s1c[1][:], axis=mybir.AxisListType.X)
        nc.vector.reduce_sum(out=arq1[:, 1:2], in_=s2c[1][:], axis=mybir.AxisListType.X)
        art1 = allreduce(arq1, 2, "1")
        for ci in range(8, NCH):
            gather_chunk(2, 2, ci)
            conv1_chain(2, ci - 8)
        for ci in range(8, NCH):
            conv1_chain(2, ci)
        arq2 = sb.tile([C, 2], F32, name="arq2")
        nc.vector.reduce_sum(out=arq2[:, 0:1], in_=s1c[2][:], axis=mybir.AxisListType.X)
        nc.vector.reduce_sum(out=arq2[:, 1:2], in_=s2c[2][:], axis=mybir.AxisListType.X)
        art2 = allreduce(arq2, 2, "2")

        # ================ conv2 + final ARs + merge ================
        def conv2_branch(br, aff):
            a_col, c_col = aff
            o1t = o1[br]
            for jt, (j0, jn) in enumerate(JT):
                o1v = o1t[:, 0 : 3 * N].rearrange("p (t n) -> p t n", t=3)[
                    :, :, j0 : j0 + jn
                ]
                nc.scalar.activation(
                    out=o1v, in_=o1v, func=AF.Relu, scale=a_col[:], bias=c_col[:]
                )
                ps = st_ps.tile([C, 512], F32, name=f"o2ps_{br}_{jt}", tag="st")
                for dd in range(3):
                    nc.tensor.matmul(
                        out=ps[:, :jn],
                        lhsT=w[br]["w2t"][:, dd * C : (dd + 1) * C],
                        rhs=o1t[:, dd * N + j0 : dd * N + j0 + jn],
                        start=(dd == 0), stop=(dd == 2),
                    )
                nc.scalar.activation(
                    out=o2[br][:, j0 : j0 + jn], in_=ps[:, :jn], func=AF.Copy,
                    accum_out=s1d[br][:, jt : jt + 1],
                )
                osq = sb.tile([C, 512], F16, name=f"o2sq_{br}_{jt}", tag="o2sq", bufs=2)
                nc.scalar.activation(
                    out=osq[:, :jn], in_=ps[:, :jn], func=AF.Square,
                    accum_out=s2d[br][:, jt : jt + 1],
                )

        aff1_1 = affine_from(art1, 0, B * N * 3, 1, (0, 1), "c1b1")
        conv2_branch(1, aff1_1)
        arq3 = sb.tile([C, 2], F32, name="arq3")
        nc.vector.reduce_sum(out=arq3[:, 0:1], in_=s1d[1][:], axis=mybir.AxisListType.X)
        nc.vector.reduce_sum(out=arq3[:, 1:2], in_=s2d[1][:], axis=mybir.AxisListType.X)
        art3 = allreduce(arq3, 2, "3")

        aff1_2 = affine_from(art2, 0, B * N * 3, 2, (0, 1), "c1b2")
        conv2_branch(2, aff1_2)
        arq4 = sb.tile([C, 2], F32, name="arq4")
        nc.vector.reduce_sum(out=arq4[:, 0:1], in_=s1d[2][:], axis=mybir.AxisListType.X)
        nc.vector.reduce_sum(out=arq4[:, 1:2], in_=s2d[2][:], axis=mybir.AxisListType.X)
        art4 = allreduce(arq4, 2, "4")

        # f1 while AR4 is in flight
        a1, c1 = affine_from(art3, 0, B * N, 1, (2, 3), "c2b1")
        for j0, jn in JT:
            nc.scalar.activation(
                out=f1t[:, j0 : j0 + jn], in_=o2[1][:, j0 : j0 + jn],
                func=AF.Relu, scale=a1[:], bias=c1[:],
            )
        a2, c2 = affine_from(art4, 0, B * N, 2, (2, 3), "c2b2")
        if delta_nonneg:
            a2d = sb.tile([C, 1], F32, name="a2d")
            nc.vector.tensor_tensor(out=a2d[:], in0=a2[:], in1=dcol[:], op=ALU.mult)
            c2d = sb.tile([C, 1], F32, name="c2d")
            nc.vector.tensor_tensor(out=c2d[:], in0=c2[:], in1=dcol[:], op=ALU.mult)
        for j0, jn in JT:
            f2 = sb.tile([C, 512], F16, name=f"f2_{j0}", tag="f2", bufs=2)
            if delta_nonneg:
                nc.scalar.activation(
                    out=f2[:, :jn], in_=o2[2][:, j0 : j0 + jn],
                    func=AF.Relu, scale=a2d[:], bias=c2d[:],
                )
            else:
                nc.scalar.activation(
                    out=f2[:, :jn], in_=o2[2][:, j0 : j0 + jn],
                    func=AF.Relu, scale=a2[:], bias=c2[:],
                )
                nc.vector.tensor_scalar_mul(f2[:, :jn], f2[:, :jn], dcol[:])
            of = sb.tile([C, 512], F32, name=f"of_{j0}", tag="of", bufs=2)
            nc.vector.tensor_tensor(
                out=of[:, :jn], in0=f1t[:, j0 : j0 + jn], in1=f2[:, :jn], op=ALU.add
            )
            nc.sync.dma_start(out=out_t[:, j0 : j0 + jn], in_=of[:, :jn])


# ======================= host side =======================

_CACHE = {}


def _prep_branch(w1, b1, g1, be1, w2, b2, g2, be2):
    w1 = np.asarray(w1, dtype=np.float32)
    w2 = np.asarray(w2, dtype=np.float32)
    A = w1[:, :C, 0, :]  # [o, i, 3]
    Bm = w1[:, C:, 0, :]  # [o, i, 3]
    P = (A + Bm).sum(axis=2)  # [o, i]
    nbt = np.ascontiguousarray(
        np.concatenate([(-Bm[:, :, d]).T for d in range(3)], axis=1)
    ).astype(np.float16)  # [i, 3C]
    utc = np.ascontiguousarray((P - Bm[:, :, 0]).T).astype(np.float16)  # u lhsT
    vtc = np.ascontiguousarray(P.T).astype(np.float16)  # v lhsT
    w2t = np.ascontiguousarray(
        np.concatenate([w2[:, :, 0, d].T for d in range(3)], axis=1)
    ).astype(np.float16)  # [i, 3C]
    bn = np.ascontiguousarray(
        np.stack(
            [
                np.asarray(g1, np.float32),
                np.asarray(be1, np.float32),
                np.asarray(g2, np.float32),
                np.asarray(be2, np.float32),
            ],
            axis=1,
        )
    )  # [C, 4]
    return nbt, utc, vtc, w2t, bn


def kernel(**inputs):
    features = np.ascontiguousarray(np.asarray(inputs["features"], np.float32))
    motion = np.ascontiguousarray(np.asarray(inputs["motion"], np.float32))
    delta = np.asarray(inputs["delta"], np.float32).reshape(-1)[0]

    nbt1, utc1, vtc1, w2t1, bn1 = _prep_branch(
        inputs["d1_w1"], inputs["d1_b1"], inputs["d1_g1"], inputs["d1_be1"],
        inputs["d1_w2"], inputs["d1_b2"], inputs["d1_g2"], inputs["d1_be2"],
    )
    nbt2, utc2, vtc2, w2t2, bn2 = _prep_branch(
        inputs["d2_w1"], inputs["d2_b1"], inputs["d2_g1"], inputs["d2_be1"],
        inputs["d2_w2"], inputs["d2_b2"], inputs["d2_g2"], inputs["d2_be2"],
    )

    delta_nonneg = bool(delta >= 0.0)
    key = ("dg3", delta_nonneg)
    if key not in _CACHE:
        _CACHE[key] = build_kernel(delta_nonneg)
    nc = _CACHE[key]

    shared = {
        "nbt1": nbt1, "utc1": utc1, "vtc1": vtc1, "w2t1": w2t1, "bn1": bn1,
        "nbt2": nbt2, "utc2": utc2, "vtc2": vtc2, "w2t2": w2t2, "bn2": bn2,
        "delta": np.array([[delta]], np.float32),
    }
    in_maps = []
    for c in range(B):
        im = dict(shared)
        im["feat"] = np.ascontiguousarray(features[c, :, :, 0])
        im["mot"] = np.ascontiguousarray(motion[c, :, :, 0])
        in_maps.append(im)

    import os

    trace = bool(int(os.environ.get("DG_KERNEL_TRACE", "0")))
    res = bass_utils.run_bass_kernel_spmd(
        nc, in_maps, core_ids=list(range(B)), trace=trace
    )
    global LAST_RESULTS
    LAST_RESULTS = res
    out = np.stack([res.results[c]["out"] for c in range(B)], axis=0)
    return out.reshape(B, C, N, 1).astype(np.float32)


LAST_RESULTS = None


# revision 14
# speedup vs baseline: 1.7586x; 1.1347x over previous
"""DG-block (dual graph-conv) Trainium2 kernel — nn_DG_Block, v3.

Reference per batch item b (B=8, C=128, N=2000, K=9):
  idx1 = top9(knn keys on features_b); idx2 = top9(... motion_b)
  gf_i = graph_feature(features_b, idx_i) -> [2C, N, 9]
  f_i  = conv_bn_relu(1x3 stride 3) -> conv_bn_relu(1x3) on gf_i
  out_b = f1 + delta * f2        [C, N, 1]
BatchNorm pools over the WHOLE batch -> stats all-reduced across cores.
Sharding: one batch item per NeuronCore (8 cores); params replicated.

Algebra (per branch; w1 [C,2C,1,3] split A_d/B_d; conv biases cancel in BN):
  conv1[o,n,t] = base_t[o,n] + sum_d y_d[idx[n,3t+d]][o],  y_d = -B_d x,
  base_0 = u = (P-B_0) x (self tap folded), base_1 = base_2 = v = P x,
  P = sum_d A_d+B_d.  knn rank key: <x_i,x_j> - |x_j|^2/2.

v3 design (baseline 501us; v2b's 938us taught the hard lessons):
  * pd matmul EXACT fp32 (input rounding -> 4.5% err, gate 2e-2). The
    -|x_j|^2/2 row accumulates INTO pd PSUM via one K=2 fp16 hi/lo
    matmul per 512-tile (residual 1.5e-5) -> kills the [C,N] negsq
    materialization + 140us of DVE/GpSimd adds.
  * PSUM: pd ([C,1024]x4bufs, 4 banks) + st (2) + tp (2) coexist ->
    no pool barriers; tables run INSIDE kNN-f so PE never drops to the
    cold p-state; scans read an SBUF pdt staging copy (ACT does the 2
    copies/chunk, ACT has the headroom).
  * value path fp16 (host-sim 5e-4 rel err; gate 2e-2).
  * gathers: 32x 1024-idx fp16 dma_gather (~4us engine-hold each,
    measured; 2048-idx and transpose-mode crash the ucode). Emitted as
    one dense stream starting mid-kNN-f (branch1 idx group A is ready
    after chunk 7) so the serial GpSimd holds fully overlap kNN.
  * conv1: point-major window sums (DVE+GpSimd) -> 3 fp16 PE
    transposes -> DVE scalar_tensor_tensor fuses the channel-major
    u/v base-add + PSUM->SBUF copy + BN-sum accumulation in 2 ops.
  * collectives: dummy warm-up AllReduce at t~0 (first CC op pays
    ~25us cold-start), 4 small ARs, only the last exposed.
"""

import numpy as np

import concourse.bacc as bacc
import concourse.bass as bass
import concourse.mybir as mybir
import concourse.tile as tile
import concourse.bass_utils as bass_utils
from concourse.masks import make_identity

F32 = mybir.dt.float32
F16 = mybir.dt.float16
U32 = mybir.dt.uint32
I16 = mybir.dt.int16
AF = mybir.ActivationFunctionType
ALU = mybir.AluOpType

B = 8
C = 128
N = 2000
EPS = 1e-5
NEG_BIG = -1.0e30

CHUNKS = [(i * 128, min(128, N - i * 128)) for i in range((N + 127) // 128)]
NCH = len(CHUNKS)  # 16
JT = [(j * 512, min(512, N - j * 512)) for j in range(4)]


def build_kernel(delta_nonneg: bool):
    nc = bacc.Bacc(
        "TRN2",
        target_bir_lowering=False,
        debug=False,
        enable_asserts=False,
        num_devices=B,
        num_swdge_queues=4,
    )

    feat_in = nc.dram_tensor("feat", [C, N], F32, kind="ExternalInput").ap()
    mot_in = nc.dram_tensor("mot", [C, N], F32, kind="ExternalInput").ap()
    wb = {}
    for br in (1, 2):
        wb[br] = {
            "nbt": nc.dram_tensor(f"nbt{br}", [C, 3 * C], F16, kind="ExternalInput").ap(),
            "utc": nc.dram_tensor(f"utc{br}", [C, C], F16, kind="ExternalInput").ap(),
            "vtc": nc.dram_tensor(f"vtc{br}", [C, C], F16, kind="ExternalInput").ap(),
            "w2t": nc.dram_tensor(f"w2t{br}", [C, 3 * C], F16, kind="ExternalInput").ap(),
            "bn": nc.dram_tensor(f"bn{br}", [C, 4], F32, kind="ExternalInput").ap(),
        }
    delta_in = nc.dram_tensor("delta", [1, 1], F32, kind="ExternalInput").ap()
    out_t = nc.dram_tensor("out", [C, N], F32, kind="ExternalOutput").ap()

    with tile.TileContext(nc) as tc:
        _emit(nc, tc, feat_in, mot_in, wb, delta_in, out_t, delta_nonneg)
    nc.compile()
    return nc


def _emit(nc, tc, feat_in, mot_in, wb, delta_in, out_t, delta_nonneg):
    import contextlib

    ctx = contextlib.ExitStack()
    with ctx:
        sb = ctx.enter_context(tc.tile_pool(name="sb", bufs=1))
        dr = ctx.enter_context(tc.tile_pool(name="dr", bufs=1, space="DRAM"))
        # pd: 2 tiles/chunk x 2 chunks in flight = 4 banks
        pd_ps = ctx.enter_context(tc.tile_pool(name="pd", bufs=4, space="PSUM"))
        st_ps = ctx.enter_context(tc.tile_pool(name="st", bufs=2, space="PSUM"))
        tp_ps = ctx.enter_context(tc.tile_pool(name="tp", bufs=2, space="PSUM"))

        # ---------------- persistent on-chip data ----------------
        x = sb.tile([C, N], F32, name="x")
        nc.sync.dma_start(out=x[:], in_=feat_in)
        m = sb.tile([C, N], F32, name="m")
        nc.sync.dma_start(out=m[:], in_=mot_in)

        ident = sb.tile([C, C], F32, name="ident")
        make_identity(nc, ident[:])
        ineg = sb.tile([C, C], F32, name="ineg")
        nc.scalar.activation(out=ineg[:], in_=ident[:], func=AF.Copy, scale=NEG_BIG)
        identh = sb.tile([C, C], F16, name="identh")
        nc.scalar.activation(out=identh[:], in_=ident[:], func=AF.Copy)
        ones1 = sb.tile([1, C], F32, name="ones1")
        nc.vector.memset(ones1[:], 1.0)
        ones2 = sb.tile([2, C], F16, name="ones2")
        nc.vector.memset(ones2[:], 1.0)
        neghalfc = sb.tile([C, 1], F32, name="neghalfc")
        nc.vector.memset(neghalfc[:], -0.5)

        w = {}
        for br in (1, 2):
            nbt = sb.tile([C, 3 * C], F16, name=f"nbt{br}")
            nc.sync.dma_start(out=nbt[:], in_=wb[br]["nbt"])
            utc = sb.tile([C, C], F16, name=f"utc{br}")
            nc.sync.dma_start(out=utc[:], in_=wb[br]["utc"])
            vtc = sb.tile([C, C], F16, name=f"vtc{br}")
            nc.sync.dma_start(out=vtc[:], in_=wb[br]["vtc"])
            w2t = sb.tile([C, 3 * C], F16, name=f"w2t{br}")
            nc.sync.dma_start(out=w2t[:], in_=wb[br]["w2t"])
            bn = sb.tile([C, 4], F32, name=f"bn{br}")
            nc.sync.dma_start(out=bn[:], in_=wb[br]["bn"])
            w[br] = dict(nbt=nbt, utc=utc, vtc=vtc, w2t=w2t, bn=bn)

        delta_sb = sb.tile([1, 1], F32, name="delta_sb")
        nc.sync.dma_start(out=delta_sb[:], in_=delta_in)

        ytab = {br: dr.tile([N, 3 * C], F16, name=f"ytab{br}") for br in (1, 2)}
        idx8 = {s: sb.tile([C, NCH * 8], U32, name=f"idx8_{s}") for s in (1, 2)}
        idx9 = {s: sb.tile([C, NCH * 8], U32, name=f"idx9_{s}") for s in (1, 2)}
        idxw = {s: sb.tile([C, NCH * 64], I16, name=f"idxw_{s}") for s in (1, 2)}
        negsq2 = {s: sb.tile([2, N], F16, name=f"negsq2_{s}") for s in (1, 2)}
        # channel-major conv1 bases: [u plane | v plane]
        uvch = {br: sb.tile([C, 2 * N], F16, name=f"uvch{br}") for br in (1, 2)}
        o1 = {br: sb.tile([C, 3 * N], F16, name=f"o1_{br}") for br in (1, 2)}
        o2 = {br: sb.tile([C, N], F16, name=f"o2_{br}") for br in (1, 2)}
        f1t = sb.tile([C, N], F16, name="f1t")

        pat8 = sb.tile([C, 8], U32, name="pat8")
        for j in range(1, 9):
            nc.vector.memset(pat8[:, j - 1 : j], j % 3)

        # conv1 stats: 2 cols/chunk (w0-op + w12-op accums); conv2: 4 JT cols
        s1c = {br: sb.tile([C, 2 * NCH], F32, name=f"s1c{br}") for br in (1, 2)}
        s2c = {br: sb.tile([C, NCH], F32, name=f"s2c{br}") for br in (1, 2)}
        s1d = {br: sb.tile([C, 4], F32, name=f"s1d{br}") for br in (1, 2)}
        s2d = {br: sb.tile([C, 4], F32, name=f"s2d{br}") for br in (1, 2)}

        # ---------------- collective helpers ----------------
        # fire_ar launches the collective; read_ar (the DRAM->SBUF readback,
        # which BLOCKS the issuing sync queue until the CC completes) is
        # emitted separately, right before the consumer, so unrelated DMAs
        # queued in between are not head-of-line blocked.
        def fire_ar(arq, k, name):
            ar_in = dr.tile([C, k], F32, name=f"arin{name}")
            ar_out = dr.tile([C, k], F32, name=f"arout{name}", addr_space="Shared")
            nc.sync.dma_start(out=ar_in[:], in_=arq[:])
            nc.gpsimd.collective_compute(
                "AllReduce",
                ALU.add,
                replica_groups=[list(range(B))],
                ins=[ar_in[:].opt()],
                outs=[ar_out[:].opt()],
            )
            return ar_out

        def read_ar(ar_out, k, name):
            art = sb.tile([C, k], F32, name=f"art{name}")
            nc.sync.dma_start(out=art[:], in_=ar_out[:])
            return art

        # warm-up: first CC op pays ~25us cold-start; burn it at t~0.
        # No readback — nobody consumes it.
        warm = sb.tile([C, 2], F32, name="warm")
        nc.vector.memset(warm[:], 0.0)
        fire_ar(warm, 2, "wu")

        def affine_from(art, col, m_count, br, bn_cols, name):
            inv_m = 1.0 / float(m_count)
            gcol = w[br]["bn"][:, bn_cols[0] : bn_cols[0] + 1]
            bcol = w[br]["bn"][:, bn_cols[1] : bn_cols[1] + 1]
            mean = sb.tile([C, 1], F32, name=f"mean{name}")
            nc.vector.tensor_scalar_mul(mean[:], art[:, col : col + 1], inv_m)
            ey2 = sb.tile([C, 1], F32, name=f"ey2{name}")
            nc.vector.tensor_scalar_mul(ey2[:], art[:, col + 1 : col + 2], inv_m)
            var = sb.tile([C, 1], F32, name=f"var{name}")
            nc.vector.tensor_tensor(out=var[:], in0=mean[:], in1=mean[:], op=ALU.mult)
            nc.vector.tensor_tensor(out=var[:], in0=ey2[:], in1=var[:], op=ALU.subtract)
            nc.vector.tensor_scalar_add(var[:], var[:], EPS)
            rv = sb.tile([C, 1], F32, name=f"rv{name}")
            nc.vector.reciprocal(rv[:], var[:])
            rstd = sb.tile([C, 1], F32, name=f"rstd{name}")
            nc.scalar.activation(out=rstd[:], in_=rv[:], func=AF.Sqrt)
            a_col = sb.tile([C, 1], F32, name=f"acol{name}")
            nc.vector.tensor_tensor(out=a_col[:], in0=gcol, in1=rstd[:], op=ALU.mult)
            c_col = sb.tile([C, 1], F32, name=f"ccol{name}")
            nc.vector.tensor_tensor(out=c_col[:], in0=mean[:], in1=a_col[:], op=ALU.mult)
            nc.vector.tensor_tensor(out=c_col[:], in0=bcol, in1=c_col[:], op=ALU.subtract)
            return (a_col, c_col)

        # ---------------- startup numerics ----------------
        dcol = sb.tile([C, 1], F32, name="dcol")
        dps = st_ps.tile([C, 8], F32, name="dps", tag="st")
        nc.tensor.matmul(
            out=dps[:, 0:1], lhsT=ones1[:], rhs=delta_sb[0:1, 0:1], start=True, stop=True
        )
        nc.scalar.activation(out=dcol[:], in_=dps[:, 0:1], func=AF.Copy)

        def negsq_prep(s, src):
            # exact fp32 row -|x_j|^2/2 -> fp16 hi/lo pair (residual ~1.5e-5)
            xsq = sb.tile([C, N], F32, name=f"xsq_{s}", tag="xsq", bufs=1)
            nc.scalar.activation(out=xsq[:], in_=src[:], func=AF.Square)
            sqrow = sb.tile([1, N], F32, name=f"sqrow_{s}", tag="sqrow", bufs=1)
            for j0, jn in JT:
                sqps = st_ps.tile([1, 512], F32, name=f"sqps_{s}_{j0}", tag="st")
                nc.tensor.matmul(
                    out=sqps[0:1, :jn], lhsT=neghalfc[:], rhs=xsq[:, j0 : j0 + jn],
                    start=True, stop=True,
                )
                nc.scalar.activation(
                    out=sqrow[0:1, j0 : j0 + jn], in_=sqps[0:1, :jn], func=AF.Copy
                )
            nc.scalar.activation(out=negsq2[s][0:1, :], in_=sqrow[0:1, :], func=AF.Copy)
            hi32 = sb.tile([1, N], F32, name=f"hi32_{s}", tag="hi32", bufs=1)
            nc.scalar.activation(out=hi32[0:1, :], in_=negsq2[s][0:1, :], func=AF.Copy)
            lo32 = sb.tile([1, N], F32, name=f"lo32_{s}", tag="lo32", bufs=1)
            nc.vector.tensor_tensor(
                out=lo32[0:1, :], in0=sqrow[0:1, :], in1=hi32[0:1, :], op=ALU.subtract
            )
            # engines can't address base partition 1; bounce via DMA
            lo16row = sb.tile([1, N], F16, name=f"lo16_{s}", tag="lo16", bufs=1)
            nc.scalar.activation(out=lo16row[0:1, :], in_=lo32[0:1, :], func=AF.Copy)
            nc.sync.dma_start(out=negsq2[s][1:2, :], in_=lo16row[0:1, :])

        negsq_prep(1, x)
        xh = sb.tile([C, N], F16, name="xh")
        nc.scalar.activation(out=xh[:], in_=x[:], func=AF.Copy)
        negsq_prep(2, m)

        # ---------------- per-chunk emitters ----------------
        def tables_chunk(br, ci):
            c0, cn = CHUNKS[ci]
            yps = st_ps.tile([C, 384], F32, name=f"yps_{br}_{ci}", tag="st")
            nc.tensor.matmul(
                out=yps[:cn, :], lhsT=xh[:, c0 : c0 + cn], rhs=w[br]["nbt"][:],
                start=True, stop=True,
            )
            yst = sb.tile([C, 384], F16, name=f"yst_{br}_{ci}", tag="yst", bufs=3)
            nc.scalar.activation(out=yst[:cn, :], in_=yps[:cn, :], func=AF.Copy)
            nc.sync.dma_start(out=ytab[br][c0 : c0 + cn, :], in_=yst[:cn, :])

        def uv_tables(br):
            # channel-major u = (P-B0)x, v = Px (bases added post-transpose)
            for which, lhsw in (("u", w[br]["utc"]), ("v", w[br]["vtc"])):
                off = 0 if which == "u" else N
                for j0, jn in JT:
                    ps = st_ps.tile([C, 512], F32, name=f"uv{br}{which}{j0}", tag="st")
                    nc.tensor.matmul(
                        out=ps[:, :jn], lhsT=lhsw[:], rhs=xh[:, j0 : j0 + jn],
                        start=True, stop=True,
                    )
                    nc.scalar.activation(
                        out=uvch[br][:, off + j0 : off + j0 + jn], in_=ps[:, :jn],
                        func=AF.Copy,
                    )

        def knn_chunk(src, which, ci):
            c0, cn = CHUNKS[ci]
            pdt = sb.tile([C, 2048], F32, name=f"pdt_{which}_{ci}", tag="pdt", bufs=2)
            for sub in range(4):
                j0, jn = JT[sub]
                pps = pd_ps.tile(
                    [C, 512], F32, name=f"pps_{which}_{ci}_{sub}", tag="pd"
                )
                nc.tensor.matmul(
                    out=pps[:cn, 0:jn],
                    lhsT=ones2[0:2, 0:cn],
                    rhs=negsq2[which][0:2, j0 : j0 + jn],
                    start=True, stop=False, skip_group_check=True,
                )
                nc.tensor.matmul(
                    out=pps[:cn, 0:jn],
                    lhsT=src[:, c0 : c0 + cn],
                    rhs=src[:, j0 : j0 + jn],
                    start=False, stop=True, skip_group_check=True,
                )
                nc.scalar.activation(
                    out=pdt[:cn, j0 : j0 + jn], in_=pps[:cn, 0:jn], func=AF.Copy
                )
            # self-exclusion: push the diagonal out of the top-8
            nc.vector.tensor_tensor(
                out=pdt[:cn, c0 : c0 + cn],
                in0=pdt[:cn, c0 : c0 + cn],
                in1=ineg[:cn, :cn],
                op=ALU.add,
            )
            v8 = sb.tile([C, 8], F32, name=f"v8_{which}_{ci}", tag="v8", bufs=2)
            nc.vector.max(out=v8[:cn], in_=pdt[:cn, 0:N])
            nc.vector.max_index(
                out=idx8[which][:cn, ci * 8 : ci * 8 + 8],
                in_max=v8[:cn],
                in_values=pdt[:cn, 0:N],
            )

        d2 = {s: dr.tile([16, NCH * 64], I16, name=f"ibounce_{s}") for s in (1, 2)}

        def build_idx9_group(which, g):
            # chunks [g*8, (g+1)*8) -> wrapped int16 idx table for the ucode
            lo, hi = g * 64, (g + 1) * 64
            v = idx9[which][:, lo:hi].rearrange("p (ci j) -> p ci j", j=8)
            i8 = idx8[which][:, lo:hi].rearrange("p (ci j) -> p ci j", j=8)
            nc.vector.tensor_scalar_mul(v, i8, 3)
            p8 = pat8[:, 0:8].rearrange("p (x j) -> p x j", x=1)
            p8b, _ = bass.broadcast_tensor_aps(p8, v)
            nc.vector.tensor_tensor(out=v, in0=v, in1=p8b, op=ALU.add)
            nc.vector.tensor_scalar_min(
                idx9[which][:, lo:hi], idx9[which][:, lo:hi], 3 * N - 1
            )
            loc = sb.tile([C, C], I16, name=f"loc_{which}_{g}", tag="loc", bufs=2)
            nc.vector.memset(loc[:, 64:128], 0)
            lo16 = idx9[which][:, lo:hi].bitcast(I16).rearrange(
                "p (c two) -> p c two", two=2
            )[:, :, 0]
            nc.vector.tensor_tensor(out=loc[:, 0:64], in0=lo16, in1=lo16, op=ALU.bypass)
            tt = sb.tile([C, C], I16, name=f"tt_{which}_{g}", tag="tt", bufs=2)
            nc.sync.dma_start_transpose(out=tt[:], in_=loc[:])
            tt2 = sb.tile([64, C], I16, name=f"tt2_{which}_{g}", tag="tt2", bufs=2)
            dstv = tt2[:, 0:C].rearrange("q (rr ph) -> q rr ph", rr=16)
            srcv = tt[0:64, 0:C].rearrange("q (ph rr) -> q ph rr", ph=8).rearrange(
                "q ph rr -> q rr ph"
            )
            nc.vector.tensor_tensor(out=dstv, in0=srcv, in1=srcv, op=ALU.bypass)
            d2s = d2[which][:, g * 512 : (g + 1) * 512]
            d2v = d2s.rearrange("rr (cj ph) -> cj rr ph", cj=64, ph=8)
            nc.sync.dma_start(
                out=d2v, in_=tt2[:, 0:C].rearrange("q (rr ph) -> q rr ph", rr=16)
            )
            for k in range(8):
                nc.sync.dma_start(
                    out=idxw[which][16 * k : 16 * k + 16, g * 512 : (g + 1) * 512],
                    in_=d2s,
                )

        g9tiles = {}

        def gather_chunk(br, which, ci):
            g9t = sb.tile([C, 1024], F16, name=f"g9_{br}_{ci}", tag="g9", bufs=16)
            ytab3 = ytab[br][:, :].rearrange("n (d c) -> (n d) c", d=3)
            nc.gpsimd.dma_gather(
                out_ap=g9t[:, 0:1024].rearrange("p (q e) -> p q e", q=8),
                in_ap=ytab3,
                idxs_ap=idxw[which][:, ci * 64 : ci * 64 + 64],
                num_idxs=1024,
                num_idxs_reg=1024,
                elem_size=C,
                queue_num=ci % 4,
            )
            g9tiles[(br, ci)] = g9t

        def conv1_chain(br, ci):
            # point-major window sums -> fp16 transposes -> fused base-add
            c0, cn = CHUNKS[ci]
            g9t = g9tiles.pop((br, ci))
            g3 = sb.tile([C, 384], F16, name=f"g3_{br}_{ci}", tag="g3", bufs=4)
            nc.vector.tensor_tensor(
                out=g3[:cn, 0:C], in0=g9t[:cn, 0:C], in1=g9t[:cn, C : 2 * C],
                op=ALU.add,
            )
            g12 = g9t[:, 256:1024].rearrange("p (t d c) -> p t d c", t=2, d=3)
            w12 = g3[:, C : 3 * C].rearrange("p (t c) -> p t c", t=2)
            nc.gpsimd.tensor_tensor(
                out=w12[:cn], in0=g12[:cn, :, 0, :], in1=g12[:cn, :, 1, :], op=ALU.add
            )
            nc.gpsimd.tensor_tensor(
                out=w12[:cn], in0=w12[:cn], in1=g12[:cn, :, 2, :], op=ALU.add
            )
            tps = tp_ps.tile([C, 384], F16, name=f"tps_{br}_{ci}", tag="tp")
            for t in range(3):
                nc.tensor.matmul(
                    out=tps[:, t * C : t * C + cn],
                    lhsT=g3[:cn, t * C : t * C + C],
                    rhs=identh[:cn, :cn],
                    is_transpose=True,
                    start=True, stop=True,
                    skip_group_check=True,
                )
            # fused: o1 = tps + base, BN-sum accum, PSUM->SBUF, in 2 DVE ops
            o1v = o1[br][:, 0 : 3 * N].rearrange("p (t n) -> p t n", t=3)
            nc.vector.scalar_tensor_tensor(
                out=o1v[:, 0, c0 : c0 + cn],
                in0=tps[:, 0:cn],
                scalar=0.0,
                in1=uvch[br][:, c0 : c0 + cn],
                op0=ALU.add, op1=ALU.add,
                accum_out=s1c[br][:, 2 * ci : 2 * ci + 1],
            )
            vsl = uvch[br][:, N + c0 : N + c0 + cn].rearrange("p (t n) -> p t n", t=1)
            w12t = o1v[:, 1:3, c0 : c0 + cn]
            vb, _ = bass.broadcast_tensor_aps(vsl, w12t)
            nc.vector.scalar_tensor_tensor(
                out=w12t,
                in0=tps[:, 0:384].rearrange("p (t n) -> p t n", t=3)[:, 1:3, :cn],
                scalar=0.0,
                in1=vb,
                op0=ALU.add, op1=ALU.add,
                accum_out=s1c[br][:, 2 * ci + 1 : 2 * ci + 2],
            )
            osq = sb.tile([C, 3 * 128], F16, name=f"osq_{br}_{ci}", tag="osq", bufs=2)
            nc.scalar.activation(
                out=osq[:, 0 : 3 * 128].rearrange("p (t n) -> p t n", t=3)[:, :, :cn],
                in_=o1v[:, :, c0 : c0 + cn],
                func=AF.Square,
                accum_out=s2c[br][:, ci : ci + 1],
            )

        # ================ emission schedule ================
        # kNN-f; tables fill early iterations (PE stays hot); branch-1
        # gathers start as soon as idx group A lands (after chunk 7).
        T1 = {0: [0, 1, 2], 1: [3, 4, 5], 2: [6, 7, 8], 3: [9, 10, 11],
              4: [12, 13], 5: [14, 15]}
        T2 = {6: [0, 1], 7: [2, 3], 8: [4, 5], 9: [6, 7], 10: [8, 9],
              11: [10, 11], 12: [12, 13], 13: [14, 15]}
        for ci in range(NCH):
            knn_chunk(x, 1, ci)
            for t in T1.get(ci, []):
                tables_chunk(1, t)
            if ci == 5:
                uv_tables(1)
            for t in T2.get(ci, []):
                tables_chunk(2, t)
            if ci == 7:
                build_idx9_group(1, 0)
            if ci == 14:
                uv_tables(2)
            if ci >= 8:
                gather_chunk(1, 1, ci - 8)
        build_idx9_group(1, 1)
        for ci in range(NCH):
            knn_chunk(m, 2, ci)
            if ci < 8:
                gather_chunk(1, 1, 8 + ci)
            if ci == 7:
                build_idx9_group(2, 0)
            if ci >= 8:
                gather_chunk(2, 2, ci - 8)
            if ci >= 2:
                conv1_chain(1, ci - 2)
        build_idx9_group(2, 1)
        for k in range(14, NCH):
            conv1_chain(1, k)
        # AR1: branch-1 conv1 stats
        arq1 = sb.tile([C, 2], F32, name="arq1")
        nc.vector.reduce_sum(out=arq1[:, 0:1], in_=s1c[1][:], axis=mybir.AxisListType.X)
        nc.vector.reduce_sum(out=arq1[:, 1:2], in_=s2c[1][:], axis=mybir.AxisListType.X)
        aro1 = fire_ar(arq1, 2, "1")
        for ci in range(8, NCH):
            gather_chunk(2, 2, ci)
            conv1_chain(2, ci - 8)
        for ci in range(8, NCH):
            conv1_chain(2, ci)
        arq2 = sb.tile([C, 2], F32, name="arq2")
        nc.vector.reduce_sum(out=arq2[:, 0:1], in_=s1c[2][:], axis=mybir.AxisListType.X)
        nc.vector.reduce_sum(out=arq2[:, 1:2], in_=s2c[2][:], axis=mybir.AxisListType.X)
        aro2 = fire_ar(arq2, 2, "2")

        # ================ conv2 + final ARs + merge ================
        def conv2_branch(br, aff):
            a_col, c_col = aff
            o1t = o1[br]
            for jt, (j0, jn) in enumerate(JT):
                o1v = o1t[:, 0 : 3 * N].rearrange("p (t n) -> p t n", t=3)[
                    :, :, j0 : j0 + jn
                ]
                nc.scalar.activation(
                    out=o1v, in_=o1v, func=AF.Relu, scale=a_col[:], bias=c_col[:]
                )
                ps = st_ps.tile([C, 512], F32, name=f"o2ps_{br}_{jt}", tag="st")
                for dd in range(3):
                    nc.tensor.matmul(
                        out=ps[:, :jn],
                        lhsT=w[br]["w2t"][:, dd * C : (dd + 1) * C],
                        rhs=o1t[:, dd * N + j0 : dd * N + j0 + jn],
                        start=(dd == 0), stop=(dd == 2),
                    )
                nc.scalar.activation(
                    out=o2[br][:, j0 : j0 + jn], in_=ps[:, :jn], func=AF.Copy,
                    accum_out=s1d[br][:, jt : jt + 1],
                )
                osq = sb.tile([C, 512], F16, name=f"o2sq_{br}_{jt}", tag="o2sq", bufs=2)
                nc.scalar.activation(
                    out=osq[:, :jn], in_=ps[:, :jn], func=AF.Square,
                    accum_out=s2d[br][:, jt : jt + 1],
                )

        art1 = read_ar(aro1, 2, "1")
        aff1_1 = affine_from(art1, 0, B * N * 3, 1, (0, 1), "c1b1")
        conv2_branch(1, aff1_1)
        arq3 = sb.tile([C, 2], F32, name="arq3")
        nc.vector.reduce_sum(out=arq3[:, 0:1], in_=s1d[1][:], axis=mybir.AxisListType.X)
        nc.vector.reduce_sum(out=arq3[:, 1:2], in_=s2d[1][:], axis=mybir.AxisListType.X)
        aro3 = fire_ar(arq3, 2, "3")

        art2 = read_ar(aro2, 2, "2")
        aff1_2 = affine_from(art2, 0, B * N * 3, 2, (0, 1), "c1b2")
        conv2_branch(2, aff1_2)
        arq4 = sb.tile([C, 2], F32, name="arq4")
        nc.vector.reduce_sum(out=arq4[:, 0:1], in_=s1d[2][:], axis=mybir.AxisListType.X)
        nc.vector.reduce_sum(out=arq4[:, 1:2], in_=s2d[2][:], axis=mybir.AxisListType.X)
        aro4 = fire_ar(arq4, 2, "4")

        # f1 while AR4 is in flight
        art3 = read_ar(aro3, 2, "3")
        a1, c1 = affine_from(art3, 0, B * N, 1, (2, 3), "c2b1")
        for j0, jn in JT:
            nc.scalar.activation(
                out=f1t[:, j0 : j0 + jn], in_=o2[1][:, j0 : j0 + jn],
                func=AF.Relu, scale=a1[:], bias=c1[:],
            )
        art4 = read_ar(aro4, 2, "4")
        a2, c2 = affine_from(art4, 0, B * N, 2, (2, 3), "c2b2")
        if delta_nonneg:
            a2d = sb.tile([C, 1], F32, name="a2d")
            nc.vector.tensor_tensor(out=a2d[:], in0=a2[:], in1=dcol[:], op=ALU.mult)
            c2d = sb.tile([C, 1], F32, name="c2d")
            nc.vector.tensor_tensor(out=c2d[:], in0=c2[:], in1=dcol[:], op=ALU.mult)
        for j0, jn in JT:
            f2 = sb.tile([C, 512], F16, name=f"f2_{j0}", tag="f2", bufs=2)
            if delta_nonneg:
                nc.scalar.activation(
                    out=f2[:, :jn], in_=o2[2][:, j0 : j0 + jn],
                    func=AF.Relu, scale=a2d[:], bias=c2d[:],
                )
            else:
                nc.scalar.activation(
                    out=f2[:, :jn], in_=o2[2][:, j0 : j0 + jn],
                    func=AF.Relu, scale=a2[:], bias=c2[:],
                )
                nc.vector.tensor_scalar_mul(f2[:, :jn], f2[:, :jn], dcol[:])
            of = sb.tile([C, 512], F32, name=f"of_{j0}", tag="of", bufs=2)
            nc.vector.tensor_tensor(
                out=of[:, :jn], in0=f1t[:, j0 : j0 + jn], in1=f2[:, :jn], op=ALU.add
            )
            nc.sync.dma_start(out=out_t[:, j0 : j0 + jn], in_=of[:, :jn])


# ======================= host side =======================

_CACHE = {}


def _prep_branch(w1, b1, g1, be1, w2, b2, g2, be2):
    w1 = np.asarray(w1, dtype=np.float32)
    w2 = np.asarray(w2, dtype=np.float32)
    A = w1[:, :C, 0, :]  # [o, i, 3]
    Bm = w1[:, C:, 0, :]  # [o, i, 3]
    P = (A + Bm).sum(axis=2)  # [o, i]
    nbt = np.ascontiguousarray(
        np.concatenate([(-Bm[:, :, d]).T for d in range(3)], axis=1)
    ).astype(np.float16)  # [i, 3C]
    utc = np.ascontiguousarray((P - Bm[:, :, 0]).T).astype(np.float16)  # u lhsT
    vtc = np.ascontiguousarray(P.T).astype(np.float16)  # v lhsT
    w2t = np.ascontiguousarray(
        np.concatenate([w2[:, :, 0, d].T for d in range(3)], axis=1)
    ).astype(np.float16)  # [i, 3C]
    bn = np.ascontiguousarray(
        np.stack(
            [
                np.asarray(g1, np.float32),
                np.asarray(be1, np.float32),
                np.asarray(g2, np.float32),
                np.asarray(be2, np.float32),
            ],
            axis=1,
        )
    )  # [C, 4]
    return nbt, utc, vtc, w2t, bn


def kernel(**inputs):
    features = np.ascontiguousarray(np.asarray(inputs["features"], np.float32))
    motion = np.ascontiguousarray(np.asarray(inputs["motion"], np.float32))
    delta = np.asarray(inputs["delta"], np.float32).reshape(-1)[0]

    nbt1, utc1, vtc1, w2t1, bn1 = _prep_branch(
        inputs["d1_w1"], inputs["d1_b1"], inputs["d1_g1"], inputs["d1_be1"],
        inputs["d1_w2"], inputs["d1_b2"], inputs["d1_g2"], inputs["d1_be2"],
    )
    nbt2, utc2, vtc2, w2t2, bn2 = _prep_branch(
        inputs["d2_w1"], inputs["d2_b1"], inputs["d2_g1"], inputs["d2_be1"],
        inputs["d2_w2"], inputs["d2_b2"], inputs["d2_g2"], inputs["d2_be2"],
    )

    delta_nonneg = bool(delta >= 0.0)
    key = ("dg3", delta_nonneg)
    if key not in _CACHE:
        _CACHE[key] = build_kernel(delta_nonneg)
    nc = _CACHE[key]

    shared = {
        "nbt1": nbt1, "utc1": utc1, "vtc1": vtc1, "w2t1": w2t1, "bn1": bn1,
        "nbt2": nbt2, "utc2": utc2, "vtc2": vtc2, "w2t2": w2t2, "bn2": bn2,
        "delta": np.array([[delta]], np.float32),
    }
    in_maps = []
    for c in range(B):
        im = dict(shared)
        im["feat"] = np.ascontiguousarray(features[c, :, :, 0])
        im["mot"] = np.ascontiguousarray(motion[c, :, :, 0])
        in_maps.append(im)

    import os

    trace = bool(int(os.environ.get("DG_KERNEL_TRACE", "0")))
    res = bass_utils.run_bass_kernel_spmd(
        nc, in_maps, core_ids=list(range(B)), trace=trace
    )
    global LAST_RESULTS
    LAST_RESULTS = res
    out = np.stack([res.results[c]["out"] for c in range(B)], axis=0)
    return out.reshape(B, C, N, 1).astype(np.float32)


LAST_RESULTS = None


# revision 15
# speedup vs baseline: 1.8108x; 1.0297x over previous
"""DG-block (dual graph-conv) Trainium2 kernel — nn_DG_Block, v3.

Reference per batch item b (B=8, C=128, N=2000, K=9):
  idx1 = top9(knn keys on features_b); idx2 = top9(... motion_b)
  gf_i = graph_feature(features_b, idx_i) -> [2C, N, 9]
  f_i  = conv_bn_relu(1x3 stride 3) -> conv_bn_relu(1x3) on gf_i
  out_b = f1 + delta * f2        [C, N, 1]
BatchNorm pools over the WHOLE batch -> stats all-reduced across cores.
Sharding: one batch item per NeuronCore (8 cores); params replicated.

Algebra (per branch; w1 [C,2C,1,3] split A_d/B_d; conv biases cancel in BN):
  conv1[o,n,t] = base_t[o,n] + sum_d y_d[idx[n,3t+d]][o],  y_d = -B_d x,
  base_0 = u = (P-B_0) x (self tap folded), base_1 = base_2 = v = P x,
  P = sum_d A_d+B_d.  knn rank key: <x_i,x_j> - |x_j|^2/2.

v3 design (baseline 501us; v2b's 938us taught the hard lessons):
  * pd matmul EXACT fp32 (input rounding -> 4.5% err, gate 2e-2). The
    -|x_j|^2/2 row accumulates INTO pd PSUM via one K=2 fp16 hi/lo
    matmul per 512-tile (residual 1.5e-5) -> kills the [C,N] negsq
    materialization + 140us of DVE/GpSimd adds.
  * PSUM: pd ([C,1024]x4bufs, 4 banks) + st (2) + tp (2) coexist ->
    no pool barriers; tables run INSIDE kNN-f so PE never drops to the
    cold p-state; scans read an SBUF pdt staging copy (ACT does the 2
    copies/chunk, ACT has the headroom).
  * value path fp16 (host-sim 5e-4 rel err; gate 2e-2).
  * gathers: 32x 1024-idx fp16 dma_gather (~4us engine-hold each,
    measured; 2048-idx and transpose-mode crash the ucode). Emitted as
    one dense stream starting mid-kNN-f (branch1 idx group A is ready
    after chunk 7) so the serial GpSimd holds fully overlap kNN.
  * conv1: point-major window sums (DVE+GpSimd) -> 3 fp16 PE
    transposes -> DVE scalar_tensor_tensor fuses the channel-major
    u/v base-add + PSUM->SBUF copy + BN-sum accumulation in 2 ops.
  * collectives: dummy warm-up AllReduce at t~0 (first CC op pays
    ~25us cold-start), 4 small ARs, only the last exposed.
"""

import numpy as np

import concourse.bacc as bacc
import concourse.bass as bass
import concourse.mybir as mybir
import concourse.tile as tile
import concourse.bass_utils as bass_utils
from concourse.masks import make_identity

F32 = mybir.dt.float32
F16 = mybir.dt.float16
U32 = mybir.dt.uint32
I16 = mybir.dt.int16
AF = mybir.ActivationFunctionType
ALU = mybir.AluOpType

B = 8
C = 128
N = 2000
EPS = 1e-5
NEG_BIG = -1.0e30

CHUNKS = [(i * 128, min(128, N - i * 128)) for i in range((N + 127) // 128)]
NCH = len(CHUNKS)  # 16
JT = [(j * 512, min(512, N - j * 512)) for j in range(4)]


def build_kernel(delta_nonneg: bool):
    nc = bacc.Bacc(
        "TRN2",
        target_bir_lowering=False,
        debug=False,
        enable_asserts=False,
        num_devices=B,
        num_swdge_queues=4,
    )

    feat_in = nc.dram_tensor("feat", [C, N], F32, kind="ExternalInput").ap()
    mot_in = nc.dram_tensor("mot", [C, N], F32, kind="ExternalInput").ap()
    wb = {}
    for br in (1, 2):
        wb[br] = {
            "nbt": nc.dram_tensor(f"nbt{br}", [C, 3 * C], F16, kind="ExternalInput").ap(),
            "utc": nc.dram_tensor(f"utc{br}", [C, C], F16, kind="ExternalInput").ap(),
            "vtc": nc.dram_tensor(f"vtc{br}", [C, C], F16, kind="ExternalInput").ap(),
            "w2t": nc.dram_tensor(f"w2t{br}", [C, 3 * C], F16, kind="ExternalInput").ap(),
            "bn": nc.dram_tensor(f"bn{br}", [C, 4], F32, kind="ExternalInput").ap(),
        }
    delta_in = nc.dram_tensor("delta", [1, 1], F32, kind="ExternalInput").ap()
    out_t = nc.dram_tensor("out", [C, N], F32, kind="ExternalOutput").ap()

    with tile.TileContext(nc) as tc:
        _emit(nc, tc, feat_in, mot_in, wb, delta_in, out_t, delta_nonneg)
    nc.compile()
    return nc


def _emit(nc, tc, feat_in, mot_in, wb, delta_in, out_t, delta_nonneg):
    import contextlib

    ctx = contextlib.ExitStack()
    with ctx:
        sb = ctx.enter_context(tc.tile_pool(name="sb", bufs=1))
        dr = ctx.enter_context(tc.tile_pool(name="dr", bufs=1, space="DRAM"))
        # pd: 2 tiles/chunk x 2 chunks in flight = 4 banks
        pd_ps = ctx.enter_context(tc.tile_pool(name="pd", bufs=4, space="PSUM"))
        st_ps = ctx.enter_context(tc.tile_pool(name="st", bufs=2, space="PSUM"))
        tp_ps = ctx.enter_context(tc.tile_pool(name="tp", bufs=2, space="PSUM"))

        # ---------------- persistent on-chip data ----------------
        x = sb.tile([C, N], F32, name="x")
        nc.sync.dma_start(out=x[:], in_=feat_in)
        m = sb.tile([C, N], F32, name="m")
        nc.sync.dma_start(out=m[:], in_=mot_in)

        ident = sb.tile([C, C], F32, name="ident")
        make_identity(nc, ident[:])
        ineg = sb.tile([C, C], F32, name="ineg")
        nc.scalar.activation(out=ineg[:], in_=ident[:], func=AF.Copy, scale=NEG_BIG)
        identh = sb.tile([C, C], F16, name="identh")
        nc.scalar.activation(out=identh[:], in_=ident[:], func=AF.Copy)
        ones1 = sb.tile([1, C], F32, name="ones1")
        nc.vector.memset(ones1[:], 1.0)
        ones2 = sb.tile([2, C], F16, name="ones2")
        nc.vector.memset(ones2[:], 1.0)
        neghalfc = sb.tile([C, 1], F32, name="neghalfc")
        nc.vector.memset(neghalfc[:], -0.5)

        w = {}
        for br in (1, 2):
            nbt = sb.tile([C, 3 * C], F16, name=f"nbt{br}")
            nc.sync.dma_start(out=nbt[:], in_=wb[br]["nbt"])
            utc = sb.tile([C, C], F16, name=f"utc{br}")
            nc.sync.dma_start(out=utc[:], in_=wb[br]["utc"])
            vtc = sb.tile([C, C], F16, name=f"vtc{br}")
            nc.sync.dma_start(out=vtc[:], in_=wb[br]["vtc"])
            w2t = sb.tile([C, 3 * C], F16, name=f"w2t{br}")
            nc.sync.dma_start(out=w2t[:], in_=wb[br]["w2t"])
            bn = sb.tile([C, 4], F32, name=f"bn{br}")
            nc.sync.dma_start(out=bn[:], in_=wb[br]["bn"])
            w[br] = dict(nbt=nbt, utc=utc, vtc=vtc, w2t=w2t, bn=bn)

        delta_sb = sb.tile([1, 1], F32, name="delta_sb")
        nc.sync.dma_start(out=delta_sb[:], in_=delta_in)

        ytab = {br: dr.tile([N, 3 * C], F16, name=f"ytab{br}") for br in (1, 2)}
        idx8 = {s: sb.tile([C, NCH * 8], U32, name=f"idx8_{s}") for s in (1, 2)}
        idx9 = {s: sb.tile([C, NCH * 8], U32, name=f"idx9_{s}") for s in (1, 2)}
        idxw = {s: sb.tile([C, NCH * 64], I16, name=f"idxw_{s}") for s in (1, 2)}
        negsq2 = {s: sb.tile([2, N], F16, name=f"negsq2_{s}") for s in (1, 2)}
        # channel-major conv1 bases: [u plane | v plane]
        uvch = {br: sb.tile([C, 2 * N], F16, name=f"uvch{br}") for br in (1, 2)}
        o1 = {br: sb.tile([C, 3 * N], F16, name=f"o1_{br}") for br in (1, 2)}
        o2 = {br: sb.tile([C, N], F16, name=f"o2_{br}") for br in (1, 2)}
        f1t = sb.tile([C, N], F16, name="f1t")

        pat8 = sb.tile([C, 8], U32, name="pat8")
        for j in range(1, 9):
            nc.vector.memset(pat8[:, j - 1 : j], j % 3)

        # conv1 stats: 2 cols/chunk (w0-op + w12-op accums); conv2: 4 JT cols
        s1c = {br: sb.tile([C, 2 * NCH], F32, name=f"s1c{br}") for br in (1, 2)}
        s2c = {br: sb.tile([C, NCH], F32, name=f"s2c{br}") for br in (1, 2)}
        s1d = {br: sb.tile([C, 4], F32, name=f"s1d{br}") for br in (1, 2)}
        s2d = {br: sb.tile([C, 4], F32, name=f"s2d{br}") for br in (1, 2)}

        # ---------------- collective helpers ----------------
        # fire_ar launches the collective; read_ar (the DRAM->SBUF readback,
        # which BLOCKS the issuing sync queue until the CC completes) is
        # emitted separately, right before the consumer, so unrelated DMAs
        # queued in between are not head-of-line blocked.
        def fire_ar(arq, k, name):
            ar_in = dr.tile([C, k], F32, name=f"arin{name}")
            ar_out = dr.tile([C, k], F32, name=f"arout{name}", addr_space="Shared")
            nc.sync.dma_start(out=ar_in[:], in_=arq[:])
            nc.gpsimd.collective_compute(
                "AllReduce",
                ALU.add,
                replica_groups=[list(range(B))],
                ins=[ar_in[:].opt()],
                outs=[ar_out[:].opt()],
            )
            return ar_out

        def read_ar(ar_out, k, name):
            art = sb.tile([C, k], F32, name=f"art{name}")
            nc.sync.dma_start(out=art[:], in_=ar_out[:])
            return art

        warm = sb.tile([C, 2], F32, name="warm")
        nc.vector.memset(warm[:], 0.0)

        def affine_from(art, col, m_count, br, bn_cols, name):
            inv_m = 1.0 / float(m_count)
            gcol = w[br]["bn"][:, bn_cols[0] : bn_cols[0] + 1]
            bcol = w[br]["bn"][:, bn_cols[1] : bn_cols[1] + 1]
            mean = sb.tile([C, 1], F32, name=f"mean{name}")
            nc.vector.tensor_scalar_mul(mean[:], art[:, col : col + 1], inv_m)
            ey2 = sb.tile([C, 1], F32, name=f"ey2{name}")
            nc.vector.tensor_scalar_mul(ey2[:], art[:, col + 1 : col + 2], inv_m)
            var = sb.tile([C, 1], F32, name=f"var{name}")
            nc.vector.tensor_tensor(out=var[:], in0=mean[:], in1=mean[:], op=ALU.mult)
            nc.vector.tensor_tensor(out=var[:], in0=ey2[:], in1=var[:], op=ALU.subtract)
            nc.vector.tensor_scalar_add(var[:], var[:], EPS)
            rv = sb.tile([C, 1], F32, name=f"rv{name}")
            nc.vector.reciprocal(rv[:], var[:])
            rstd = sb.tile([C, 1], F32, name=f"rstd{name}")
            nc.scalar.activation(out=rstd[:], in_=rv[:], func=AF.Sqrt)
            a_col = sb.tile([C, 1], F32, name=f"acol{name}")
            nc.vector.tensor_tensor(out=a_col[:], in0=gcol, in1=rstd[:], op=ALU.mult)
            c_col = sb.tile([C, 1], F32, name=f"ccol{name}")
            nc.vector.tensor_tensor(out=c_col[:], in0=mean[:], in1=a_col[:], op=ALU.mult)
            nc.vector.tensor_tensor(out=c_col[:], in0=bcol, in1=c_col[:], op=ALU.subtract)
            return (a_col, c_col)

        # ---------------- startup numerics ----------------
        dcol = sb.tile([C, 1], F32, name="dcol")
        dps = st_ps.tile([C, 8], F32, name="dps", tag="st")
        nc.tensor.matmul(
            out=dps[:, 0:1], lhsT=ones1[:], rhs=delta_sb[0:1, 0:1], start=True, stop=True
        )
        nc.scalar.activation(out=dcol[:], in_=dps[:, 0:1], func=AF.Copy)

        def negsq_prep(s, src):
            # exact fp32 row -|x_j|^2/2 -> fp16 hi/lo pair (residual ~1.5e-5)
            xsq = sb.tile([C, N], F32, name=f"xsq_{s}", tag="xsq", bufs=1)
            nc.scalar.activation(out=xsq[:], in_=src[:], func=AF.Square)
            sqrow = sb.tile([1, N], F32, name=f"sqrow_{s}", tag="sqrow", bufs=1)
            for j0, jn in JT:
                sqps = st_ps.tile([1, 512], F32, name=f"sqps_{s}_{j0}", tag="st")
                nc.tensor.matmul(
                    out=sqps[0:1, :jn], lhsT=neghalfc[:], rhs=xsq[:, j0 : j0 + jn],
                    start=True, stop=True,
                )
                nc.scalar.activation(
                    out=sqrow[0:1, j0 : j0 + jn], in_=sqps[0:1, :jn], func=AF.Copy
                )
            nc.scalar.activation(out=negsq2[s][0:1, :], in_=sqrow[0:1, :], func=AF.Copy)
            hi32 = sb.tile([1, N], F32, name=f"hi32_{s}", tag="hi32", bufs=1)
            nc.scalar.activation(out=hi32[0:1, :], in_=negsq2[s][0:1, :], func=AF.Copy)
            lo32 = sb.tile([1, N], F32, name=f"lo32_{s}", tag="lo32", bufs=1)
            nc.vector.tensor_tensor(
                out=lo32[0:1, :], in0=sqrow[0:1, :], in1=hi32[0:1, :], op=ALU.subtract
            )
            # engines can't address base partition 1; bounce via DMA
            lo16row = sb.tile([1, N], F16, name=f"lo16_{s}", tag="lo16", bufs=1)
            nc.scalar.activation(out=lo16row[0:1, :], in_=lo32[0:1, :], func=AF.Copy)
            nc.sync.dma_start(out=negsq2[s][1:2, :], in_=lo16row[0:1, :])

        negsq_prep(1, x)
        xh = sb.tile([C, N], F16, name="xh")
        nc.scalar.activation(out=xh[:], in_=x[:], func=AF.Copy)
        negsq_prep(2, m)

        # ---------------- per-chunk emitters ----------------
        def tables_chunk(br, ci):
            c0, cn = CHUNKS[ci]
            yps = st_ps.tile([C, 384], F32, name=f"yps_{br}_{ci}", tag="st")
            nc.tensor.matmul(
                out=yps[:cn, :], lhsT=xh[:, c0 : c0 + cn], rhs=w[br]["nbt"][:],
                start=True, stop=True,
            )
            yst = sb.tile([C, 384], F16, name=f"yst_{br}_{ci}", tag="yst", bufs=3)
            nc.scalar.activation(out=yst[:cn, :], in_=yps[:cn, :], func=AF.Copy)
            # second HWDGE queue (ACT): keeps the bulky ytab writes from
            # delaying the tiny idxw DMAs the gathers wait on
            nc.scalar.dma_start(out=ytab[br][c0 : c0 + cn, :], in_=yst[:cn, :])

        def uv_tables(br):
            # channel-major u = (P-B0)x, v = Px (bases added post-transpose)
            for which, lhsw in (("u", w[br]["utc"]), ("v", w[br]["vtc"])):
                off = 0 if which == "u" else N
                for j0, jn in JT:
                    ps = st_ps.tile([C, 512], F32, name=f"uv{br}{which}{j0}", tag="st")
                    nc.tensor.matmul(
                        out=ps[:, :jn], lhsT=lhsw[:], rhs=xh[:, j0 : j0 + jn],
                        start=True, stop=True,
                    )
                    nc.scalar.activation(
                        out=uvch[br][:, off + j0 : off + j0 + jn], in_=ps[:, :jn],
                        func=AF.Copy,
                    )

        def knn_chunk(src, which, ci):
            c0, cn = CHUNKS[ci]
            pdt = sb.tile([C, 2048], F32, name=f"pdt_{which}_{ci}", tag="pdt", bufs=2)
            for sub in range(4):
                j0, jn = JT[sub]
                pps = pd_ps.tile(
                    [C, 512], F32, name=f"pps_{which}_{ci}_{sub}", tag="pd"
                )
                nc.tensor.matmul(
                    out=pps[:cn, 0:jn],
                    lhsT=ones2[0:2, 0:cn],
                    rhs=negsq2[which][0:2, j0 : j0 + jn],
                    start=True, stop=False, skip_group_check=True,
                )
                nc.tensor.matmul(
                    out=pps[:cn, 0:jn],
                    lhsT=src[:, c0 : c0 + cn],
                    rhs=src[:, j0 : j0 + jn],
                    start=False, stop=True, skip_group_check=True,
                )
                nc.scalar.activation(
                    out=pdt[:cn, j0 : j0 + jn], in_=pps[:cn, 0:jn], func=AF.Copy
                )
            # self-exclusion: push the diagonal out of the top-8
            nc.vector.tensor_tensor(
                out=pdt[:cn, c0 : c0 + cn],
                in0=pdt[:cn, c0 : c0 + cn],
                in1=ineg[:cn, :cn],
                op=ALU.add,
            )
            v8 = sb.tile([C, 8], F32, name=f"v8_{which}_{ci}", tag="v8", bufs=2)
            nc.vector.max(out=v8[:cn], in_=pdt[:cn, 0:N])
            nc.vector.max_index(
                out=idx8[which][:cn, ci * 8 : ci * 8 + 8],
                in_max=v8[:cn],
                in_values=pdt[:cn, 0:N],
            )

        d2 = {s: dr.tile([16, NCH * 64], I16, name=f"ibounce_{s}") for s in (1, 2)}

        def build_idx9_group(which, g):
            # chunks [g*8, (g+1)*8) -> wrapped int16 idx table for the ucode
            lo, hi = g * 64, (g + 1) * 64
            v = idx9[which][:, lo:hi].rearrange("p (ci j) -> p ci j", j=8)
            i8 = idx8[which][:, lo:hi].rearrange("p (ci j) -> p ci j", j=8)
            nc.vector.tensor_scalar_mul(v, i8, 3)
            p8 = pat8[:, 0:8].rearrange("p (x j) -> p x j", x=1)
            p8b, _ = bass.broadcast_tensor_aps(p8, v)
            nc.vector.tensor_tensor(out=v, in0=v, in1=p8b, op=ALU.add)
            nc.vector.tensor_scalar_min(
                idx9[which][:, lo:hi], idx9[which][:, lo:hi], 3 * N - 1
            )
            loc = sb.tile([C, C], I16, name=f"loc_{which}_{g}", tag="loc", bufs=2)
            nc.vector.memset(loc[:, 64:128], 0)
            lo16 = idx9[which][:, lo:hi].bitcast(I16).rearrange(
                "p (c two) -> p c two", two=2
            )[:, :, 0]
            nc.vector.tensor_tensor(out=loc[:, 0:64], in0=lo16, in1=lo16, op=ALU.bypass)
            tt = sb.tile([C, C], I16, name=f"tt_{which}_{g}", tag="tt", bufs=2)
            nc.sync.dma_start_transpose(out=tt[:], in_=loc[:])
            tt2 = sb.tile([64, C], I16, name=f"tt2_{which}_{g}", tag="tt2", bufs=2)
            dstv = tt2[:, 0:C].rearrange("q (rr ph) -> q rr ph", rr=16)
            srcv = tt[0:64, 0:C].rearrange("q (ph rr) -> q ph rr", ph=8).rearrange(
                "q ph rr -> q rr ph"
            )
            nc.vector.tensor_tensor(out=dstv, in0=srcv, in1=srcv, op=ALU.bypass)
            d2s = d2[which][:, g * 512 : (g + 1) * 512]
            d2v = d2s.rearrange("rr (cj ph) -> cj rr ph", cj=64, ph=8)
            nc.sync.dma_start(
                out=d2v, in_=tt2[:, 0:C].rearrange("q (rr ph) -> q rr ph", rr=16)
            )
            for k in range(8):
                nc.sync.dma_start(
                    out=idxw[which][16 * k : 16 * k + 16, g * 512 : (g + 1) * 512],
                    in_=d2s,
                )

        g9tiles = {}

        def gather_chunk(br, which, ci):
            g9t = sb.tile([C, 1024], F16, name=f"g9_{br}_{ci}", tag="g9", bufs=16)
            ytab3 = ytab[br][:, :].rearrange("n (d c) -> (n d) c", d=3)
            nc.gpsimd.dma_gather(
                out_ap=g9t[:, 0:1024].rearrange("p (q e) -> p q e", q=8),
                in_ap=ytab3,
                idxs_ap=idxw[which][:, ci * 64 : ci * 64 + 64],
                num_idxs=1024,
                num_idxs_reg=1024,
                elem_size=C,
                queue_num=ci % 4,
            )
            g9tiles[(br, ci)] = g9t

        def conv1_chain(br, ci):
            # point-major window sums -> fp16 transposes -> fused base-add
            c0, cn = CHUNKS[ci]
            g9t = g9tiles.pop((br, ci))
            g3 = sb.tile([C, 384], F16, name=f"g3_{br}_{ci}", tag="g3", bufs=4)
            nc.vector.tensor_tensor(
                out=g3[:cn, 0:C], in0=g9t[:cn, 0:C], in1=g9t[:cn, C : 2 * C],
                op=ALU.add,
            )
            g12 = g9t[:, 256:1024].rearrange("p (t d c) -> p t d c", t=2, d=3)
            w12 = g3[:, C : 3 * C].rearrange("p (t c) -> p t c", t=2)
            nc.gpsimd.tensor_tensor(
                out=w12[:cn], in0=g12[:cn, :, 0, :], in1=g12[:cn, :, 1, :], op=ALU.add
            )
            nc.gpsimd.tensor_tensor(
                out=w12[:cn], in0=w12[:cn], in1=g12[:cn, :, 2, :], op=ALU.add
            )
            tps = tp_ps.tile([C, 384], F16, name=f"tps_{br}_{ci}", tag="tp")
            for t in range(3):
                nc.tensor.matmul(
                    out=tps[:, t * C : t * C + cn],
                    lhsT=g3[:cn, t * C : t * C + C],
                    rhs=identh[:cn, :cn],
                    is_transpose=True,
                    start=True, stop=True,
                    skip_group_check=True,
                )
            # fused: o1 = tps + base, BN-sum accum, PSUM->SBUF, in 2 DVE ops
            o1v = o1[br][:, 0 : 3 * N].rearrange("p (t n) -> p t n", t=3)
            nc.vector.scalar_tensor_tensor(
                out=o1v[:, 0, c0 : c0 + cn],
                in0=tps[:, 0:cn],
                scalar=0.0,
                in1=uvch[br][:, c0 : c0 + cn],
                op0=ALU.add, op1=ALU.add,
                accum_out=s1c[br][:, 2 * ci : 2 * ci + 1],
            )
            vsl = uvch[br][:, N + c0 : N + c0 + cn].rearrange("p (t n) -> p t n", t=1)
            w12t = o1v[:, 1:3, c0 : c0 + cn]
            vb, _ = bass.broadcast_tensor_aps(vsl, w12t)
            nc.vector.scalar_tensor_tensor(
                out=w12t,
                in0=tps[:, 0:384].rearrange("p (t n) -> p t n", t=3)[:, 1:3, :cn],
                scalar=0.0,
                in1=vb,
                op0=ALU.add, op1=ALU.add,
                accum_out=s1c[br][:, 2 * ci + 1 : 2 * ci + 2],
            )
            osq = sb.tile([C, 3 * 128], F16, name=f"osq_{br}_{ci}", tag="osq", bufs=2)
            nc.scalar.activation(
                out=osq[:, 0 : 3 * 128].rearrange("p (t n) -> p t n", t=3)[:, :, :cn],
                in_=o1v[:, :, c0 : c0 + cn],
                func=AF.Square,
                accum_out=s2c[br][:, ci : ci + 1],
            )

        # ================ emission schedule ================
        # kNN-f; tables fill early iterations (PE stays hot); branch-1
        # gathers start as soon as idx group A lands (after chunk 7).
        T1 = {0: [0, 1, 2], 1: [3, 4, 5], 2: [6, 7, 8], 3: [9, 10, 11],
              4: [12, 13], 5: [14, 15]}
        T2 = {6: [0, 1], 7: [2, 3], 8: [4, 5], 9: [6, 7], 10: [8, 9],
              11: [10, 11], 12: [12, 13], 13: [14, 15]}
        for ci in range(NCH):
            knn_chunk(x, 1, ci)
            for t in T1.get(ci, []):
                tables_chunk(1, t)
            if ci == 5:
                uv_tables(1)
            for t in T2.get(ci, []):
                tables_chunk(2, t)
            if ci == 7:
                build_idx9_group(1, 0)
            if ci == 14:
                uv_tables(2)
            if ci >= 8:
                gather_chunk(1, 1, ci - 8)
        build_idx9_group(1, 1)
        # warm-up AR here: the cold CC setup (~25us) runs while branch-1
        # group-B gathers wait on their idx build anyway. No readback.
        fire_ar(warm, 2, "wu")
        for ci in range(NCH):
            knn_chunk(m, 2, ci)
            if ci < 8:
                gather_chunk(1, 1, 8 + ci)
            if ci == 7:
                build_idx9_group(2, 0)
            if ci >= 8:
                gather_chunk(2, 2, ci - 8)
            # chains lag their gathers by >=6 chunks so the GpSimd w12 ops
            # never wait on in-flight gather data (head-of-line on the queue)
            if ci >= 6:
                conv1_chain(1, ci - 6)
        build_idx9_group(2, 1)
        for k in range(10, NCH):
            conv1_chain(1, k)
        # AR1: branch-1 conv1 stats
        arq1 = sb.tile([C, 2], F32, name="arq1")
        nc.vector.reduce_sum(out=arq1[:, 0:1], in_=s1c[1][:], axis=mybir.AxisListType.X)
        nc.vector.reduce_sum(out=arq1[:, 1:2], in_=s2c[1][:], axis=mybir.AxisListType.X)
        aro1 = fire_ar(arq1, 2, "1")
        for ci in range(8, NCH):
            gather_chunk(2, 2, ci)
        for ci in range(NCH):
            conv1_chain(2, ci)
        arq2 = sb.tile([C, 2], F32, name="arq2")
        nc.vector.reduce_sum(out=arq2[:, 0:1], in_=s1c[2][:], axis=mybir.AxisListType.X)
        nc.vector.reduce_sum(out=arq2[:, 1:2], in_=s2c[2][:], axis=mybir.AxisListType.X)
        aro2 = fire_ar(arq2, 2, "2")

        # ================ conv2 + final ARs + merge ================
        def conv2_branch(br, aff):
            a_col, c_col = aff
            o1t = o1[br]
            for jt, (j0, jn) in enumerate(JT):
                o1v = o1t[:, 0 : 3 * N].rearrange("p (t n) -> p t n", t=3)[
                    :, :, j0 : j0 + jn
                ]
                nc.scalar.activation(
                    out=o1v, in_=o1v, func=AF.Relu, scale=a_col[:], bias=c_col[:]
                )
                ps = st_ps.tile([C, 512], F32, name=f"o2ps_{br}_{jt}", tag="st")
                for dd in range(3):
                    nc.tensor.matmul(
                        out=ps[:, :jn],
                        lhsT=w[br]["w2t"][:, dd * C : (dd + 1) * C],
                        rhs=o1t[:, dd * N + j0 : dd * N + j0 + jn],
                        start=(dd == 0), stop=(dd == 2),
                    )
                nc.scalar.activation(
                    out=o2[br][:, j0 : j0 + jn], in_=ps[:, :jn], func=AF.Copy,
                    accum_out=s1d[br][:, jt : jt + 1],
                )
                osq = sb.tile([C, 512], F16, name=f"o2sq_{br}_{jt}", tag="o2sq", bufs=2)
                nc.scalar.activation(
                    out=osq[:, :jn], in_=ps[:, :jn], func=AF.Square,
                    accum_out=s2d[br][:, jt : jt + 1],
                )

        art1 = read_ar(aro1, 2, "1")
        aff1_1 = affine_from(art1, 0, B * N * 3, 1, (0, 1), "c1b1")
        conv2_branch(1, aff1_1)
        arq3 = sb.tile([C, 2], F32, name="arq3")
        nc.vector.reduce_sum(out=arq3[:, 0:1], in_=s1d[1][:], axis=mybir.AxisListType.X)
        nc.vector.reduce_sum(out=arq3[:, 1:2], in_=s2d[1][:], axis=mybir.AxisListType.X)
        aro3 = fire_ar(arq3, 2, "3")

        art2 = read_ar(aro2, 2, "2")
        aff1_2 = affine_from(art2, 0, B * N * 3, 2, (0, 1), "c1b2")
        conv2_branch(2, aff1_2)
        arq4 = sb.tile([C, 2], F32, name="arq4")
        nc.vector.reduce_sum(out=arq4[:, 0:1], in_=s1d[2][:], axis=mybir.AxisListType.X)
        nc.vector.reduce_sum(out=arq4[:, 1:2], in_=s2d[2][:], axis=mybir.AxisListType.X)
        aro4 = fire_ar(arq4, 2, "4")

        # f1 while AR4 is in flight
        art3 = read_ar(aro3, 2, "3")
        a1, c1 = affine_from(art3, 0, B * N, 1, (2, 3), "c2b1")
        for j0, jn in JT:
            nc.scalar.activation(
                out=f1t[:, j0 : j0 + jn], in_=o2[1][:, j0 : j0 + jn],
                func=AF.Relu, scale=a1[:], bias=c1[:],
            )
        art4 = read_ar(aro4, 2, "4")
        a2, c2 = affine_from(art4, 0, B * N, 2, (2, 3), "c2b2")
        if delta_nonneg:
            a2d = sb.tile([C, 1], F32, name="a2d")
            nc.vector.tensor_tensor(out=a2d[:], in0=a2[:], in1=dcol[:], op=ALU.mult)
            c2d = sb.tile([C, 1], F32, name="c2d")
            nc.vector.tensor_tensor(out=c2d[:], in0=c2[:], in1=dcol[:], op=ALU.mult)
        for j0, jn in JT:
            f2 = sb.tile([C, 512], F16, name=f"f2_{j0}", tag="f2", bufs=2)
            if delta_nonneg:
                nc.scalar.activation(
                    out=f2[:, :jn], in_=o2[2][:, j0 : j0 + jn],
                    func=AF.Relu, scale=a2d[:], bias=c2d[:],
                )
            else:
                nc.scalar.activation(
                    out=f2[:, :jn], in_=o2[2][:, j0 : j0 + jn],
                    func=AF.Relu, scale=a2[:], bias=c2[:],
                )
                nc.vector.tensor_scalar_mul(f2[:, :jn], f2[:, :jn], dcol[:])
            of = sb.tile([C, 512], F32, name=f"of_{j0}", tag="of", bufs=2)
            nc.vector.tensor_tensor(
                out=of[:, :jn], in0=f1t[:, j0 : j0 + jn], in1=f2[:, :jn], op=ALU.add
            )
            nc.sync.dma_start(out=out_t[:, j0 : j0 + jn], in_=of[:, :jn])


# ======================= host side =======================

_CACHE = {}


def _prep_branch(w1, b1, g1, be1, w2, b2, g2, be2):
    w1 = np.asarray(w1, dtype=np.float32)
    w2 = np.asarray(w2, dtype=np.float32)
    A = w1[:, :C, 0, :]  # [o, i, 3]
    Bm = w1[:, C:, 0, :]  # [o, i, 3]
    P = (A + Bm).sum(axis=2)  # [o, i]
    nbt = np.ascontiguousarray(
        np.concatenate([(-Bm[:, :, d]).T for d in range(3)], axis=1)
    ).astype(np.float16)  # [i, 3C]
    utc = np.ascontiguousarray((P - Bm[:, :, 0]).T).astype(np.float16)  # u lhsT
    vtc = np.ascontiguousarray(P.T).astype(np.float16)  # v lhsT
    w2t = np.ascontiguousarray(
        np.concatenate([w2[:, :, 0, d].T for d in range(3)], axis=1)
    ).astype(np.float16)  # [i, 3C]
    bn = np.ascontiguousarray(
        np.stack(
            [
                np.asarray(g1, np.float32),
                np.asarray(be1, np.float32),
                np.asarray(g2, np.float32),
                np.asarray(be2, np.float32),
            ],
            axis=1,
        )
    )  # [C, 4]
    return nbt, utc, vtc, w2t, bn


def kernel(**inputs):
    features = np.ascontiguousarray(np.asarray(inputs["features"], np.float32))
    motion = np.ascontiguousarray(np.asarray(inputs["motion"], np.float32))
    delta = np.asarray(inputs["delta"], np.float32).reshape(-1)[0]

    nbt1, utc1, vtc1, w2t1, bn1 = _prep_branch(
        inputs["d1_w1"], inputs["d1_b1"], inputs["d1_g1"], inputs["d1_be1"],
        inputs["d1_w2"], inputs["d1_b2"], inputs["d1_g2"], inputs["d1_be2"],
    )
    nbt2, utc2, vtc2, w2t2, bn2 = _prep_branch(
        inputs["d2_w1"], inputs["d2_b1"], inputs["d2_g1"], inputs["d2_be1"],
        inputs["d2_w2"], inputs["d2_b2"], inputs["d2_g2"], inputs["d2_be2"],
    )

    delta_nonneg = bool(delta >= 0.0)
    key = ("dg3", delta_nonneg)
    if key not in _CACHE:
        _CACHE[key] = build_kernel(delta_nonneg)
    nc = _CACHE[key]

    shared = {
        "nbt1": nbt1, "utc1": utc1, "vtc1": vtc1, "w2t1": w2t1, "bn1": bn1,
        "nbt2": nbt2, "utc2": utc2, "vtc2": vtc2, "w2t2": w2t2, "bn2": bn2,
        "delta": np.array([[delta]], np.float32),
    }
    in_maps = []
    for c in range(B):
        im = dict(shared)
        im["feat"] = np.ascontiguousarray(features[c, :, :, 0])
        im["mot"] = np.ascontiguousarray(motion[c, :, :, 0])
        in_maps.append(im)

    import os

    trace = bool(int(os.environ.get("DG_KERNEL_TRACE", "0")))
    res = bass_utils.run_bass_kernel_spmd(
        nc, in_maps, core_ids=list(range(B)), trace=trace
    )
    global LAST_RESULTS
    LAST_RESULTS = res
    out = np.stack([res.results[c]["out"] for c in range(B)], axis=0)
    return out.reshape(B, C, N, 1).astype(np.float32)


LAST_RESULTS = None
